# revision 26
# baseline (speedup 1.0000x reference)
"""CapsuleLayer (dynamic routing) on 8 trn2 NeuronCores — v2.

Math: u_hat[b,c,i,o] = sum_{d,k} W[c,0,i,o,d,k] x[b,i,k]
             = sum_k Wsum[c,i,o,k] x[b,i,k],  Wsum = W.sum(d)   (134MB -> 8.4MB)
Sharded over IN_CAPS (i) across 8 cores; only s-partials cross cores.

v2 redesign vs v1 (which was vector-engine bound at 419us):
  * s lives packed by class in PSUM tiles at PE-quadrant-legal bases; the
    AllReduce ships 2-class quarters (bf16, 16KB) that pipeline with compute
    and with the next iteration's start.
  * wf layout [i, (h,k,c,o)] serves all consumers: class-paired s0
    stationaries [128,32] (contiguous), per-class s stationaries [128,16],
    and 8 per-k PE transposes per chunk into TokA/TokB whose k-blocks sit
    at 32-aligned slot bases (16 valid + 16 dead rows per slot).
  * w4 = running sum of squash outputs v, stored slot-replicated
    [128=(4 slots x (16o+16 dead)), (c,b)] via two tiny PE replication
    matmuls per class — so every G matmul has lhsT (Tok slot) and rhs
    (w4 slot) at the same 32-aligned base.
  * d-reduce = one DVE tensor_reduce over d per chunk (Pool add-tree for the
    late classes so DVE is free for overlapped work).
  * k-fold of P = x*G is one DVE tensor_reduce per 4-k half chunk.
  * softmax: iter1 skips max-subtraction (|logit| <~ 65); 1/den is folded
    into x (xs = x * recb) so e stays unnormalized and ct is never formed.
  * h-major iteration order: softmax(h0) overlaps the P-phase of h1.
"""

import contextlib
import sys
import types

import numpy as np
import ml_dtypes  # noqa: F401


def _install_ntff_shim():
    try:
        import antenv.axon_hooks  # noqa: F401

        return
    except Exception:
        pass
    import ctypes

    mod = types.ModuleType("antenv.axon_hooks")
    holder = [None, False]

    def set_axon_ntff_profile_hook(h):
        holder[0], holder[1] = h, True

    def _make_hook():
        try:
            lib = ctypes.CDLL("/opt/axon/libaxon_pjrt.so")
        except OSError:
            return None
        if not hasattr(lib, "axon_start_nrt_profile"):
            return None
        lib.axon_start_nrt_profile.argtypes = [
            ctypes.POINTER(ctypes.c_int64),
            ctypes.c_size_t,
        ]
        lib.axon_start_nrt_profile.restype = ctypes.c_int64
        lib.axon_stop_nrt_profile.argtypes = [ctypes.c_char_p]
        lib.axon_stop_nrt_profile.restype = ctypes.c_int64

        @contextlib.contextmanager
        def _hook(output_dir, device_ids):
            import jax

            jax.devices()
            if device_ids:
                ids = (ctypes.c_int64 * len(device_ids))(*device_ids)
                rc = lib.axon_start_nrt_profile(ids, len(device_ids))
            else:
                rc = lib.axon_start_nrt_profile(None, 0)
            if rc != 0:
                raise RuntimeError(f"axon_start_nrt_profile rc={rc}")
            try:
                yield
            finally:
                n = lib.axon_stop_nrt_profile(str(output_dir).encode())
                print(
                    f"profile: {n} file(s) written to {output_dir}",
                    file=sys.stderr,
                )

        return _hook

    def get_axon_ntff_profile_hook():
        if not holder[1]:
            holder[0], holder[1] = _make_hook(), True
        return holder[0]

    mod.set_axon_ntff_profile_hook = set_axon_ntff_profile_hook
    mod.get_axon_ntff_profile_hook = get_axon_ntff_profile_hook
    sys.modules["antenv.axon_hooks"] = mod


try:
    _install_ntff_shim()
except Exception:
    pass

import concourse.bass as bass
import concourse.mybir as mybir
import concourse.tile as tile
from concourse import masks
from concourse.bass_utils import run_bass_kernel_spmd
from bass_rust import ScopedClock

# ---------------------------------------------------------------- constants
C, I, O, D, K, B = 8, 2048, 16, 16, 8, 256
NCORES = 8
IL = I // NCORES          # 256 i's per core
F32 = mybir.dt.float32
F32R = mybir.dt.float32r
BF16 = mybir.dt.bfloat16
KB_ = K * B               # 2048 cols per h-block of x
CB = C * B                # 2048

# ------------------------------------------------- tile tail-drain workaround
_MAX_WAITS = 1


def _patched_drain_and_barrier(self, tick_clock, wait_clock):
    nc = self.nc
    drain_inst = nc.sync.drain()
    wait_clock.add_sem_waits(
        drain_inst.ins, ScopedClock({None: tick_clock.global_clock})
    )
    si = drain_inst.ins.sync_info
    if si is not None and si.on_wait and len(si.on_wait) > _MAX_WAITS:
        waits = list(si.on_wait)
        si.on_wait = waits[:_MAX_WAITS]
        for i in range(_MAX_WAITS, len(waits), _MAX_WAITS):
            extra = nc.sync.drain()
            extra.ins.sync_info = mybir.SyncInfo(
                on_wait=waits[i : i + _MAX_WAITS], on_update=[]
            )
    nc.all_engine_barrier()
    assert self.sems is not None
    popped = nc._tile_sem_poison_stack.pop()
    assert popped is self._sem_poison
    nc.clear_and_free_semaphores(list(self.sems.allocated().values()))
    nc.all_engine_barrier()


tile.TileContext._drain_and_barrier = _patched_drain_and_barrier

_fix_ctr = [0]


def fixup_multi_waits(nc):
    """walrus in this toolchain accepts at most one sem wait per instruction;
    hoist extra waits onto same-engine drains placed just before."""
    for f in nc.m.functions:
        for bb in f.blocks:
            out = []
            for inst in bb.instructions:
                si = inst.sync_info
                if si is not None and si.on_wait and len(si.on_wait) > _MAX_WAITS:
                    waits = list(si.on_wait)
                    for i in range(0, len(waits) - _MAX_WAITS, _MAX_WAITS):
                        _fix_ctr[0] += 1
                        d = mybir.InstDrain(
                            name=f"waitsplit_{_fix_ctr[0]}", ins=[], outs=[]
                        )
                        d.engine = inst.engine
                        d.sync_info = mybir.SyncInfo(
                            on_wait=waits[i : i + _MAX_WAITS], on_update=[]
                        )
                        out.append(d)
                    si.on_wait = waits[len(waits) - _MAX_WAITS :]
                out.append(inst)
            bb.instructions[:] = out
    return nc


def build_all(fixup=True):
    nc = bass.Bass("TRN2", target_bir_lowering=False, debug=False,
                   num_devices=NCORES)
    W_d = nc.dram_tensor("W", [C, IL, O, D, K], F32, kind="ExternalInput").ap()
    x_d = nc.dram_tensor("x", [B, IL, K], F32, kind="ExternalInput").ap()
    # v packed [(c,o)=128, b=256]
    v_d = nc.dram_tensor("v", [C * O, B], F32R, kind="ExternalOutput").ap()
    xt_d = nc.dram_tensor("xt", [IL * K, B], BF16).ap()
    # per-(iter, quarter) collective buffers: [16o, (2 classes, b)] bf16
    cc_in = [[nc.dram_tensor(f"cc_in{t}_{q}", [O, 2 * B], BF16).ap()
              for q in range(4)] for t in range(3)]
    cc_out = [[nc.dram_tensor(f"cc_out{t}_{q}", [O, 2 * B], BF16).ap()
               for q in range(4)] for t in range(3)]

    with tile.TileContext(nc) as tc:
        with (
            tc.tile_pool(name="const", bufs=1) as constp,
            tc.tile_pool(name="persist", bufs=1) as pers,
            tc.tile_pool(name="small", bufs=4) as smallp,
        ):
            # ---------------- constants
            ident = constp.tile([128, 128], F32)
            masks.make_identity(nc, ident[:])
            identb = constp.tile([128, 128], BF16)
            with nc.allow_low_precision(reason="identity copy"):
                nc.vector.tensor_copy(identb[:], ident[:])
            # squash reducers: sum over o (partition dim) and broadcast back
            ones16 = constp.tile([O, 1], BF16)
            nc.gpsimd.memset(ones16[:], 1.0)
            ones1 = constp.tile([1, O], BF16)
            nc.gpsimd.memset(ones1[:], 1.0)
            # ---------------- persistent state
            # xt: [i(h-blocked 128p), (h, k, b)] bf16
            xt = pers.tile([128, 2 * KB_], BF16)
            # wf: d-reduced W, [i, (h, c, k, o)] bf16
            wf = pers.tile([128, 2 * K * C * O], BF16)
            # Tok4{A,B}: [(4k, o)=64, (c, h, i128)] bf16 (G stationaries;
            # A holds k=0..3, B k=4..7)
            Tok4A = pers.tile([64, C * 2 * 128], BF16)
            Tok4B = pers.tile([64, C * 2 * 128], BF16)
            # w4q: block-diagonal mover for G: [64=(kg,o), (c, kmod4, b)];
            # diagonal [16,256] blocks hold the running v sum, rest is 0.
            # (diag blocks at partition 16m are written via SBUF-SBUF DMA —
            # engine APs must start at 32-aligned partitions, DMAs need not.)
            w4q = pers.tile([64, C * 4 * B], BF16)
            nc.gpsimd.memset(w4q[:], 0.0)
            # vacc: running v sum, [16o, (c, b)]
            vacc = pers.tile([O, C * B], BF16)
            # logits bt: [i, (h, c, b)] bf16
            bt = pers.tile([128, 2 * CB], BF16)
            # e = exp(bt - max) (unnormalized), same layout
            e_all = pers.tile([128, 2 * CB], BF16)
            # xs = x * (1/den), [i, (h, k, b)] bf16
            xs = pers.tile([128, 2 * KB_], BF16)

            wfv = wf[:].rearrange("p (h c k o) -> p h c k o", h=2, c=C, k=K,
                                  o=O)

            # ---------- AllReduce + squash on a 2-class quarter ----------
            # s arrives as [16o, (cc, b)] bf16.  scale = sqrt(ss)/(1+ss).
            def squash_quarter(t, q, pre, sqp, sqps):
                nm = f"{t}_{q}"
                B2 = 2 * B
                nc.gpsimd.collective_compute(
                    "AllReduce",
                    mybir.AluOpType.add,
                    replica_groups=[list(range(NCORES))],
                    ins=[cc_in[t][q].opt()],
                    outs=[cc_out[t][q].opt()],
                )
                s_q = sqp.tile([O, B2], BF16, tag="s_q", name=f"sq{nm}")
                nc.sync.dma_start(s_q[:], cc_out[t][q][:, :])
                sq2 = sqp.tile([O, B2], BF16, tag="sq2", name=f"sq2{nm}")
                with nc.allow_low_precision(reason="square bf16"):
                    nc.scalar.activation(
                        sq2[:], s_q[:], mybir.ActivationFunctionType.Square,
                        scale=pre,
                    )
                ssq_ps = sqps.tile([1, B2], F32, tag="ssq", name=f"ssqp{nm}")
                nc.tensor.matmul(ssq_ps[:], ones16[:], sq2[:],
                                 start=True, stop=True)
                # scale = pre * sqrt(ss) / (1+ss)
                rt = sqp.tile([1, B2], F32, tag="rt", name=f"rt{nm}")
                nc.scalar.sqrt(rt[:], ssq_ps[:])
                den1 = sqp.tile([1, B2], F32, tag="den1", name=f"den1{nm}")
                nc.vector.tensor_scalar_add(den1[:], ssq_ps[:], 1.0)
                rcp = sqp.tile([1, B2], F32, tag="rcp", name=f"rcp{nm}")
                nc.vector.reciprocal(rcp[:], den1[:])
                scl = sqp.tile([1, B2], BF16, tag="scl", name=f"scl{nm}")
                with nc.allow_low_precision(reason="scale bf16"):
                    if pre != 1.0:
                        sclf = sqp.tile([1, B2], F32, tag="sclf",
                                        name=f"sclf{nm}")
                        nc.vector.tensor_mul(sclf[:], rt[:], rcp[:])
                        nc.vector.tensor_scalar_mul(scl[:], sclf[:], pre)
                    else:
                        nc.vector.tensor_mul(scl[:], rt[:], rcp[:])
                sbc_ps = sqps.tile([O, B2], F32, tag="sbc", name=f"sbc{nm}")
                nc.tensor.matmul(sbc_ps[:], ones1[:], scl[:],
                                 start=True, stop=True)
                if t == 2:
                    v32 = sqp.tile([O, B2], F32R, tag="v32", name=f"v32{nm}")
                    with nc.allow_low_precision(reason="v out"):
                        nc.vector.tensor_mul(v32[:], s_q[:], sbc_ps[:])
                    nc.sync.dma_start(
                        v_d[32 * q : 32 * (q + 1), :].rearrange(
                            "(cc o) b -> o cc b", cc=2
                        ),
                        v32[:].rearrange("o (cc b) -> o cc b", cc=2),
                    )
                else:
                    vac = vacc[:, 2 * q * B : 2 * (q + 1) * B]
                    with nc.allow_low_precision(reason="v bf16"):
                        if t == 0:
                            nc.vector.tensor_mul(vac, s_q[:], sbc_ps[:])
                        else:
                            vq = sqp.tile([O, B2], BF16, tag="vq",
                                          name=f"vq{nm}")
                            nc.vector.tensor_mul(vq[:], s_q[:], sbc_ps[:])
                            nc.vector.tensor_add(vac, vac, vq[:])
                    # scatter into w4q's diagonal blocks via SBUF-SBUF DMA
                    # (partition-16m targets are illegal for engine APs)
                    engs = [nc.sync, nc.scalar, nc.gpsimd, nc.sync]
                    for r in range(2):
                        c = 2 * q + r
                        src = vacc[:, c * B : (c + 1) * B]
                        for m in range(4):
                            engs[m].dma_start(
                                w4q[O * m : O * (m + 1),
                                    c * 4 * B + m * B : c * 4 * B
                                    + (m + 1) * B],
                                src,
                            )

            # ---------------- phase A: x -> xt_d -> xt  (early, gpsimd DMAs)
            phio_cm = contextlib.ExitStack()
            phio = phio_cm.enter_context(tc.tile_pool(name="phio", bufs=3))
            wtree = phio_cm.enter_context(tc.tile_pool(name="wtree", bufs=2))
            with tc.tile_pool(name="xps", bufs=4, space="PSUM") as xps:
                for bh in range(2):
                    xin = phio.tile([128, IL * K], F32, tag="xin", bufs=2)
                    nc.sync.dma_start(
                        xin[:],
                        x_d[bh * 128 : (bh + 1) * 128].rearrange(
                            "b i k -> b (i k)"
                        ),
                    )
                    xc = phio.tile([128, IL * K], BF16, tag="xc", bufs=1)
                    for qq in range(16):
                        ps = xps.tile([128, 128], F32)
                        nc.tensor.transpose(
                            ps[:], xin[:, qq * 128 : (qq + 1) * 128], ident[:]
                        )
                        nc.scalar.copy(xc[:, qq * 128 : (qq + 1) * 128], ps[:])
                    nc.gpsimd.dma_start(
                        xt_d.rearrange("(q p) b -> p q b", p=128)[
                            :, :, bh * 128 : (bh + 1) * 128
                        ],
                        xc[:].rearrange("p (q b2) -> p q b2", q=16),
                    )
            for h in range(2):
                nc.gpsimd.dma_start(
                    xt[:, h * KB_ : (h + 1) * KB_],
                    xt_d[h * 1024 : (h + 1) * 1024].rearrange(
                        "(p k) b -> p (k b)", k=K
                    ),
                )

            # ---------------- phase B: W pipeline, c-major; s0 and AR0
            # quarters launch as soon as each class pair is done.
            with (
                tc.tile_pool(name="tps", bufs=2, space="PSUM") as tpsp,
                tc.tile_pool(name="s0ps", bufs=2, space="PSUM") as s0psp,
                tc.tile_pool(name="sqps0", bufs=1, space="PSUM") as sqps0,
                tc.tile_pool(name="sq0", bufs=2) as sqp0,
            ):
                s0c_ps = {}
                for c in range(C):
                    s0c_ps[c] = s0psp.tile([O, B], F32, tag="s0",
                                           name=f"s0_{c}")
                    for h in range(2):
                        wt = phio.tile([128, O * D * K], F32, tag="wt", bufs=2)
                        (nc.sync if h == 0 else nc.scalar).dma_start(
                            wt[:],
                            W_d[c, h * 128 : (h + 1) * 128].rearrange(
                                "p o d k -> p (o d k)"
                            ),
                        )
                        # ---- d-reduce into wf[:, (h, :, c, :)]
                        wout = wfv[:, h, c, :, :]  # [p, k, o] contiguous
                        win = wt[:].rearrange("p (o d k) -> p k o d", o=O,
                                              d=D, k=K)
                        if c < 4:
                            with nc.allow_low_precision(reason="wsum bf16"):
                                nc.vector.tensor_reduce(
                                    wout, win, mybir.AxisListType.X,
                                    mybir.AluOpType.add,
                                )
                        else:
                            # Pool add-tree (frees DVE for overlapped work)
                            v4 = wt[:].rearrange("p (o d k) -> p o d k", o=O,
                                                 d=D, k=K)
                            a1 = wtree.tile([128, 1024], F32, tag="a1")
                            a1v = a1[:].rearrange("p (o d k) -> p o d k",
                                                  o=O, d=8, k=K)
                            nc.gpsimd.tensor_add(a1v, v4[:, :, 0:8, :],
                                                 v4[:, :, 8:16, :])
                            a2 = wtree.tile([128, 512], F32, tag="a2")
                            a2v = a2[:].rearrange("p (o d k) -> p o d k",
                                                  o=O, d=4, k=K)
                            nc.gpsimd.tensor_add(a2v, a1v[:, :, 0:4, :],
                                                 a1v[:, :, 4:8, :])
                            a3 = wtree.tile([128, 256], F32, tag="a3")
                            a3v = a3[:].rearrange("p (o d k) -> p o d k",
                                                  o=O, d=2, k=K)
                            nc.gpsimd.tensor_add(a3v, a2v[:, :, 0:2, :],
                                                 a2v[:, :, 2:4, :])
                            wout_odk = wout.rearrange(
                                "p k o -> p o k"
                            ).unsqueeze(2)
                            with nc.allow_low_precision(reason="wsum bf16"):
                                nc.gpsimd.tensor_add(
                                    wout_odk, a3v[:, :, 0:1, :],
                                    a3v[:, :, 1:2, :]
                                )
                        # ---- batched transpose -> Tok4A/Tok4B halves
                        tcol = (h * C + c) * 128
                        tp = tpsp.tile([128, 128], BF16, tag="tp")
                        nc.tensor.transpose(
                            tp[:], wf[:, tcol : tcol + 128], identb[:]
                        )
                        nc.scalar.copy(Tok4A[:, tcol : tcol + 128],
                                       tp[0:64, :])
                        nc.scalar.copy(Tok4B[:, tcol : tcol + 128],
                                       tp[64:128, :])
                        # ---- s0 partials: 8 accumulating matmuls (k)
                        for k in range(K):
                            nc.tensor.matmul(
                                s0c_ps[c][:],
                                wfv[:, h, c, k, :],
                                xt[:, h * KB_ + k * B : h * KB_ + (k + 1) * B],
                                start=(h == 0 and k == 0),
                                stop=(h == 1 and k == K - 1),
                            )
                    if c % 2 == 1:
                        q = c // 2
                        s0q = smallp.tile([O, 2 * B], BF16, tag="s0q",
                                          name=f"s0q{q}", bufs=2)
                        with nc.allow_low_precision(reason="s partial bf16"):
                            nc.scalar.copy(s0q[:, 0:B], s0c_ps[c - 1][:])
                            nc.scalar.copy(s0q[:, B : 2 * B], s0c_ps[c][:])
                        nc.sync.dma_start(cc_in[0][q][:, :], s0q[:])
                        squash_quarter(0, q, 1.0 / C, sqp0, sqps0)

            phio_cm.close()

            # ---------------- routing iterations 1 and 2
            with (
                tc.tile_pool(name="gps", bufs=2, space="PSUM") as gps,
                tc.tile_pool(name="sps", bufs=2, space="PSUM") as spsp,
                tc.tile_pool(name="sqpsi", bufs=1, space="PSUM") as sqpsi,
                tc.tile_pool(name="workp", bufs=1) as workp,
                tc.tile_pool(name="sqi", bufs=2) as sqpi,
            ):
                for it in range(1, 3):
                    # ---- phase 1: G = Tok^T w4; P = x*G; bt = sum_k P
                    for h in range(2):
                        for c in range(C):
                            tcol = (h * C + c) * 128
                            bthc = bt[:, h * CB + c * B : h * CB + (c + 1) * B]
                            ftmp = workp.tile([128, 2 * B], BF16, tag="ftmp",
                                              bufs=3, name=f"ft{it}_{h}_{c}")
                            for kh in range(2):
                                Tok = Tok4A if kh == 0 else Tok4B
                                g_ps = gps.tile([128, 4 * B], F32, tag="g")
                                for hf in range(2):
                                    nc.tensor.matmul(
                                        g_ps[:, hf * 2 * B : (hf + 1) * 2 * B],
                                        Tok[:, tcol : tcol + 128],
                                        w4q[:, c * 4 * B + hf * 2 * B :
                                            c * 4 * B + (hf + 1) * 2 * B],
                                        start=True, stop=True,
                                    )
                                xsl = xt[:, h * KB_ + kh * 4 * B :
                                         h * KB_ + (kh + 1) * 4 * B]
                                # P = x * G : DVE reads PSUM f32 directly on
                                # h1 chunks; Act-copy + Pool mul on h0.
                                phalf = workp.tile([128, 4 * B], BF16,
                                                   tag="phalf", bufs=4,
                                                   name=f"ph{it}_{h}_{c}_{kh}")
                                with nc.allow_low_precision(reason="P bf16"):
                                    if h == 0:
                                        g16 = workp.tile(
                                            [128, 4 * B], BF16, tag="g16",
                                            bufs=3, name=f"g16{it}_{c}_{kh}",
                                        )
                                        nc.scalar.copy(g16[:], g_ps[:])
                                        nc.gpsimd.tensor_mul(
                                            phalf[:], xsl, g16[:]
                                        )
                                    else:
                                        nc.vector.tensor_mul(
                                            phalf[:], xsl, g_ps[:]
                                        )
                                # fold k (4) in one reduce
                                with nc.allow_low_precision(reason="bt bf16"):
                                    nc.vector.tensor_reduce(
                                        ftmp[:, kh * B : (kh + 1) * B],
                                        phalf[:].rearrange(
                                            "p (k b) -> p b k", k=4
                                        ),
                                        mybir.AxisListType.X,
                                        mybir.AluOpType.add,
                                    )
                            with nc.allow_low_precision(reason="bt bf16"):
                                nc.vector.tensor_add(
                                    bthc, ftmp[:, 0:B], ftmp[:, B : 2 * B]
                                )

                        # ---- phase 2 (per h): softmax over c -> e, xs
                        bth = bt[:, h * CB : (h + 1) * CB]
                        bthv = bth.rearrange("p (c b) -> p c b", c=C)
                        eh = e_all[:, h * CB : (h + 1) * CB]
                        ein = bth
                        if True:
                            # max-subtraction: the Act Exp table misbehaves
                            # for large positive inputs, so always subtract
                            m1 = workp.tile([128, 4 * B], BF16, tag="m1",
                                            bufs=2, name=f"m1_{it}_{h}")
                            m1v = m1[:].rearrange("p (c b) -> p c b", c=4)
                            m2 = workp.tile([128, 2 * B], BF16, tag="m2",
                                            bufs=2, name=f"m2_{it}_{h}")
                            m2v = m2[:].rearrange("p (c b) -> p c b", c=2)
                            rmax = workp.tile([128, B], BF16, tag="rmax",
                                              bufs=2, name=f"rm_{it}_{h}")
                            sub = workp.tile([128, CB], BF16, tag="sub",
                                             bufs=2, name=f"sub_{it}_{h}")
                            with nc.allow_low_precision(reason="softmax"):
                                nc.vector.tensor_max(
                                    m1v, bthv[:, 0:4, :], bthv[:, 4:8, :]
                                )
                                nc.vector.tensor_max(
                                    m2v, m1v[:, 0:2, :], m1v[:, 2:4, :]
                                )
                                nc.vector.tensor_max(
                                    rmax[:].unsqueeze(1),
                                    m2v[:, 0:1, :], m2v[:, 1:2, :]
                                )
                                nc.vector.tensor_sub(
                                    sub[:].rearrange("p (c b) -> p c b", c=C),
                                    bthv,
                                    rmax[:].unsqueeze(1)
                                    .broadcast_to([128, C, B]),
                                )
                            ein = sub[:]
                        nc.scalar.activation(
                            eh, ein, mybir.ActivationFunctionType.Exp
                        )
                        ehv = eh.rearrange("p (c b) -> p c b", c=C)
                        # den tree + reciprocal + xs = x * recb
                        d1 = workp.tile([128, 4 * B], BF16, tag="m1",
                                        bufs=2, name=f"d1_{it}_{h}")
                        d1v = d1[:].rearrange("p (c b) -> p c b", c=4)
                        d2 = workp.tile([128, 2 * B], BF16, tag="m2",
                                        bufs=2, name=f"d2_{it}_{h}")
                        d2v = d2[:].rearrange("p (c b) -> p c b", c=2)
                        den = workp.tile([128, B], F32, tag="den",
                                         bufs=2, name=f"den_{it}_{h}")
                        with nc.allow_low_precision(reason="den bf16"):
                            nc.vector.tensor_add(
                                d1v, ehv[:, 0:4, :], ehv[:, 4:8, :]
                            )
                            nc.vector.tensor_add(
                                d2v, d1v[:, 0:2, :], d1v[:, 2:4, :]
                            )
                        nc.vector.tensor_add(
                            den[:].unsqueeze(1),
                            d2v[:, 0:1, :], d2v[:, 1:2, :]
                        )
                        rec = workp.tile([128, B], F32, tag="rec",
                                         bufs=2, name=f"rec_{it}_{h}")
                        nc.vector.reciprocal(rec[:], den[:])
                        recb = workp.tile([128, B], BF16, tag="recb",
                                          bufs=2, name=f"recb_{it}_{h}")
                        with nc.allow_low_precision(reason="recb bf16"):
                            nc.vector.tensor_copy(recb[:], rec[:])
                            nc.vector.tensor_mul(
                                xs[:, h * KB_ : (h + 1) * KB_].rearrange(
                                    "p (k b) -> p k b", k=K
                                ),
                                xt[:, h * KB_ : (h + 1) * KB_].rearrange(
                                    "p (k b) -> p k b", k=K
                                ),
                                recb[:].unsqueeze(1)
                                .broadcast_to([128, K, B]),
                            )

                    # ---- phase 3: y = e_c * xs; s = sum wf^T y; AR quarters
                    sc_ps = {}
                    for c in range(C):
                        sc_ps[c] = spsp.tile([O, B], F32, tag="s",
                                             name=f"s{it}_{c}")
                        for h in range(2):
                            y = workp.tile([128, KB_], BF16, tag="y",
                                           bufs=3, name=f"y{it}_{c}_{h}")
                            yeng = nc.gpsimd if h == 1 else nc.vector
                            with nc.allow_low_precision(reason="y bf16"):
                                yeng.tensor_mul(
                                    y[:].rearrange("p (k b) -> p k b", k=K),
                                    xs[:, h * KB_ : (h + 1) * KB_].rearrange(
                                        "p (k b) -> p k b", k=K
                                    ),
                                    e_all[:, h * CB + c * B :
                                          h * CB + (c + 1) * B]
                                    .unsqueeze(1)
                                    .broadcast_to([128, K, B]),
                                )
                            for k in range(K):
                                nc.tensor.matmul(
                                    sc_ps[c][:],
                                    wfv[:, h, c, k, :],
                                    y[:, k * B : (k + 1) * B],
                                    start=(h == 0 and k == 0),
                                    stop=(h == 1 and k == K - 1),
                                )
                        if c % 2 == 1:
                            q = c // 2
                            sq_sb = smallp.tile([O, 2 * B], BF16,
                                                tag="sq_sb",
                                                name=f"sqsb{it}_{q}", bufs=2)
                            with nc.allow_low_precision(reason="s bf16"):
                                nc.scalar.copy(sq_sb[:, 0:B],
                                               sc_ps[c - 1][:])
                                nc.scalar.copy(sq_sb[:, B : 2 * B],
                                               sc_ps[c][:])
                            nc.sync.dma_start(cc_in[it][q][:, :], sq_sb[:])
                            squash_quarter(it, q, 1.0, sqpi, sqpsi)
    return fixup_multi_waits(nc) if fixup else nc


_NC = None


def kernel(x: np.ndarray, W: np.ndarray, _timings=None) -> np.ndarray:
    global _NC
    x = np.asarray(x, np.float32)
    W = np.asarray(W, np.float32)
    if _NC is None:
        _NC = build_all()
    in_maps = []
    for j in range(NCORES):
        sl = slice(j * IL, (j + 1) * IL)
        in_maps.append(
            {
                "W": np.ascontiguousarray(W[:, 0, sl]),
                "x": np.ascontiguousarray(x[:, sl, :]),
            }
        )
    res = run_bass_kernel_spmd(
        _NC, in_maps, core_ids=list(range(NCORES)),
        trace=_timings is not None,
    )
    if _timings is not None:
        _timings.append(res.exec_time_ns)
    v = res.results[0]["v"].astype(np.float32)  # [(c,o), b]
    return np.ascontiguousarray(
        v.reshape(C, O, B).transpose(2, 0, 1)
    )


# revision 27
# speedup vs baseline: 1.1405x; 1.1405x over previous
"""CapsuleLayer (dynamic routing) on 8 trn2 NeuronCores — v2.

Math: u_hat[b,c,i,o] = sum_{d,k} W[c,0,i,o,d,k] x[b,i,k]
             = sum_k Wsum[c,i,o,k] x[b,i,k],  Wsum = W.sum(d)   (134MB -> 8.4MB)
Sharded over IN_CAPS (i) across 8 cores; only s-partials cross cores.

v2 redesign vs v1 (which was vector-engine bound at 419us):
  * s lives packed by class in PSUM tiles at PE-quadrant-legal bases; the
    AllReduce ships 2-class quarters (bf16, 16KB) that pipeline with compute
    and with the next iteration's start.
  * wf layout [i, (h,k,c,o)] serves all consumers: class-paired s0
    stationaries [128,32] (contiguous), per-class s stationaries [128,16],
    and 8 per-k PE transposes per chunk into TokA/TokB whose k-blocks sit
    at 32-aligned slot bases (16 valid + 16 dead rows per slot).
  * w4 = running sum of squash outputs v, stored slot-replicated
    [128=(4 slots x (16o+16 dead)), (c,b)] via two tiny PE replication
    matmuls per class — so every G matmul has lhsT (Tok slot) and rhs
    (w4 slot) at the same 32-aligned base.
  * d-reduce = one DVE tensor_reduce over d per chunk (Pool add-tree for the
    late classes so DVE is free for overlapped work).
  * k-fold of P = x*G is one DVE tensor_reduce per 4-k half chunk.
  * softmax: iter1 skips max-subtraction (|logit| <~ 65); 1/den is folded
    into x (xs = x * recb) so e stays unnormalized and ct is never formed.
  * h-major iteration order: softmax(h0) overlaps the P-phase of h1.
"""

import contextlib
import sys
import types

import numpy as np
import ml_dtypes  # noqa: F401


def _install_ntff_shim():
    try:
        import antenv.axon_hooks  # noqa: F401

        return
    except Exception:
        pass
    import ctypes

    mod = types.ModuleType("antenv.axon_hooks")
    holder = [None, False]

    def set_axon_ntff_profile_hook(h):
        holder[0], holder[1] = h, True

    def _make_hook():
        try:
            lib = ctypes.CDLL("/opt/axon/libaxon_pjrt.so")
        except OSError:
            return None
        if not hasattr(lib, "axon_start_nrt_profile"):
            return None
        lib.axon_start_nrt_profile.argtypes = [
            ctypes.POINTER(ctypes.c_int64),
            ctypes.c_size_t,
        ]
        lib.axon_start_nrt_profile.restype = ctypes.c_int64
        lib.axon_stop_nrt_profile.argtypes = [ctypes.c_char_p]
        lib.axon_stop_nrt_profile.restype = ctypes.c_int64

        @contextlib.contextmanager
        def _hook(output_dir, device_ids):
            import jax

            jax.devices()
            if device_ids:
                ids = (ctypes.c_int64 * len(device_ids))(*device_ids)
                rc = lib.axon_start_nrt_profile(ids, len(device_ids))
            else:
                rc = lib.axon_start_nrt_profile(None, 0)
            if rc != 0:
                raise RuntimeError(f"axon_start_nrt_profile rc={rc}")
            try:
                yield
            finally:
                n = lib.axon_stop_nrt_profile(str(output_dir).encode())
                print(
                    f"profile: {n} file(s) written to {output_dir}",
                    file=sys.stderr,
                )

        return _hook

    def get_axon_ntff_profile_hook():
        if not holder[1]:
            holder[0], holder[1] = _make_hook(), True
        return holder[0]

    mod.set_axon_ntff_profile_hook = set_axon_ntff_profile_hook
    mod.get_axon_ntff_profile_hook = get_axon_ntff_profile_hook
    sys.modules["antenv.axon_hooks"] = mod


try:
    _install_ntff_shim()
except Exception:
    pass

import concourse.bass as bass
import concourse.mybir as mybir
import concourse.tile as tile
from concourse import masks
from concourse.bass_utils import run_bass_kernel_spmd
from bass_rust import ScopedClock

# ---------------------------------------------------------------- constants
C, I, O, D, K, B = 8, 2048, 16, 16, 8, 256
NCORES = 8
IL = I // NCORES          # 256 i's per core
F32 = mybir.dt.float32
F32R = mybir.dt.float32r
BF16 = mybir.dt.bfloat16
KB_ = K * B               # 2048 cols per h-block of x
CB = C * B                # 2048

# ------------------------------------------------- tile tail-drain workaround
_MAX_WAITS = 1


def _patched_drain_and_barrier(self, tick_clock, wait_clock):
    nc = self.nc
    drain_inst = nc.sync.drain()
    wait_clock.add_sem_waits(
        drain_inst.ins, ScopedClock({None: tick_clock.global_clock})
    )
    si = drain_inst.ins.sync_info
    if si is not None and si.on_wait and len(si.on_wait) > _MAX_WAITS:
        waits = list(si.on_wait)
        si.on_wait = waits[:_MAX_WAITS]
        for i in range(_MAX_WAITS, len(waits), _MAX_WAITS):
            extra = nc.sync.drain()
            extra.ins.sync_info = mybir.SyncInfo(
                on_wait=waits[i : i + _MAX_WAITS], on_update=[]
            )
    nc.all_engine_barrier()
    assert self.sems is not None
    popped = nc._tile_sem_poison_stack.pop()
    assert popped is self._sem_poison
    nc.clear_and_free_semaphores(list(self.sems.allocated().values()))
    nc.all_engine_barrier()


tile.TileContext._drain_and_barrier = _patched_drain_and_barrier

_fix_ctr = [0]


def fixup_multi_waits(nc):
    """walrus in this toolchain accepts at most one sem wait per instruction;
    hoist extra waits onto same-engine drains placed just before."""
    for f in nc.m.functions:
        for bb in f.blocks:
            out = []
            for inst in bb.instructions:
                si = inst.sync_info
                if si is not None and si.on_wait and len(si.on_wait) > _MAX_WAITS:
                    waits = list(si.on_wait)
                    for i in range(0, len(waits) - _MAX_WAITS, _MAX_WAITS):
                        _fix_ctr[0] += 1
                        d = mybir.InstDrain(
                            name=f"waitsplit_{_fix_ctr[0]}", ins=[], outs=[]
                        )
                        d.engine = inst.engine
                        d.sync_info = mybir.SyncInfo(
                            on_wait=waits[i : i + _MAX_WAITS], on_update=[]
                        )
                        out.append(d)
                    si.on_wait = waits[len(waits) - _MAX_WAITS :]
                out.append(inst)
            bb.instructions[:] = out
    return nc


def build_all(fixup=True):
    nc = bass.Bass("TRN2", target_bir_lowering=False, debug=False,
                   num_devices=NCORES)
    W_d = nc.dram_tensor("W", [C, IL, O, D, K], F32, kind="ExternalInput").ap()
    x_d = nc.dram_tensor("x", [B, IL, K], F32, kind="ExternalInput").ap()
    # v packed [(c,o)=128, b=256]
    v_d = nc.dram_tensor("v", [C * O, B], F32R, kind="ExternalOutput").ap()
    xt_d = nc.dram_tensor("xt", [IL * K, B], BF16).ap()
    # per-(iter, quarter) collective buffers: [16o, (2 classes, b)] bf16
    cc_in = [[nc.dram_tensor(f"cc_in{t}_{q}", [O, 2 * B], BF16).ap()
              for q in range(4)] for t in range(3)]
    cc_out = [[nc.dram_tensor(f"cc_out{t}_{q}", [O, 2 * B], BF16).ap()
               for q in range(4)] for t in range(3)]

    with tile.TileContext(nc) as tc:
        with (
            tc.tile_pool(name="const", bufs=1) as constp,
            tc.tile_pool(name="persist", bufs=1) as pers,
            tc.tile_pool(name="small", bufs=4) as smallp,
        ):
            # ---------------- constants
            ident = constp.tile([128, 128], F32)
            masks.make_identity(nc, ident[:])
            identb = constp.tile([128, 128], BF16)
            with nc.allow_low_precision(reason="identity copy"):
                nc.vector.tensor_copy(identb[:], ident[:])
            # squash reducers: sum over o (partition dim) and broadcast back
            ones16 = constp.tile([O, 1], BF16)
            nc.vector.memset(ones16[:], 1.0)
            ones1 = constp.tile([1, O], BF16)
            nc.vector.memset(ones1[:], 1.0)
            # ---------------- persistent state
            # xt: [i(h-blocked 128p), (h, k, b)] bf16
            xt = pers.tile([128, 2 * KB_], BF16)
            # wf: d-reduced W, [i, (h, c, k, o)] bf16
            wf = pers.tile([128, 2 * K * C * O], BF16)
            # Tok4{A,B}: [(4k, o)=64, (c, h, i128)] bf16 (G stationaries;
            # A holds k=0..3, B k=4..7)
            Tok4A = pers.tile([64, C * 2 * 128], BF16)
            Tok4B = pers.tile([64, C * 2 * 128], BF16)
            # w4q: block-diagonal mover for G: [64=(kg,o), (c, kmod4, b)];
            # diagonal [16,256] blocks hold the running v sum, rest is 0.
            # (diag blocks at partition 16m are written via SBUF-SBUF DMA —
            # engine APs must start at 32-aligned partitions, DMAs need not.)
            w4q = pers.tile([64, C * 4 * B], BF16)
            nc.vector.memset(w4q[:], 0.0)
            # vacc: running v sum, [16o, (c, b)]
            vacc = pers.tile([O, C * B], BF16)
            # logits bt: [i, (h, c, b)] bf16
            bt = pers.tile([128, 2 * CB], BF16)
            # e = exp(bt - max) (unnormalized), same layout
            e_all = pers.tile([128, 2 * CB], BF16)
            # xs = x * (1/den), [i, (h, k, b)] bf16
            xs = pers.tile([128, 2 * KB_], BF16)

            wfv = wf[:].rearrange("p (h c k o) -> p h c k o", h=2, c=C, k=K,
                                  o=O)

            # ---------- AllReduce + squash on a 2-class quarter ----------
            # s arrives as [16o, (cc, b)] bf16.  scale = sqrt(ss)/(1+ss).
            def squash_quarter(t, q, pre, sqp, sqps):
                nm = f"{t}_{q}"
                B2 = 2 * B
                nc.gpsimd.collective_compute(
                    "AllReduce",
                    mybir.AluOpType.add,
                    replica_groups=[list(range(NCORES))],
                    ins=[cc_in[t][q].opt()],
                    outs=[cc_out[t][q].opt()],
                )
                s_q = sqp.tile([O, B2], BF16, tag="s_q", name=f"sq{nm}")
                nc.sync.dma_start(s_q[:], cc_out[t][q][:, :])
                sq2 = sqp.tile([O, B2], BF16, tag="sq2", name=f"sq2{nm}")
                with nc.allow_low_precision(reason="square bf16"):
                    nc.scalar.activation(
                        sq2[:], s_q[:], mybir.ActivationFunctionType.Square,
                        scale=pre,
                    )
                ssq_ps = sqps.tile([1, B2], F32, tag="ssq", name=f"ssqp{nm}")
                nc.tensor.matmul(ssq_ps[:], ones16[:], sq2[:],
                                 start=True, stop=True)
                # scale = pre * sqrt(ss)/(1+ss); single-partition DVE
                # ops run one lane, so bounce [1,512] -> [128,4] via DMA.
                ssq_row = sqp.tile([1, B2], F32R, tag="ssq_row",
                                   name=f"ssqr{nm}")
                nc.scalar.copy(ssq_row[:], ssq_ps[:])
                ssq128 = sqp.tile([128, 4], F32R, tag="ssq128",
                                  name=f"ssq128{nm}")
                nc.sync.dma_start(
                    ssq128[:], ssq_row[:].rearrange("u (p f) -> u p f", p=128)
                )
                rt = sqp.tile([128, 4], F32, tag="rt", name=f"rt{nm}")
                nc.scalar.sqrt(rt[:], ssq128[:])
                den1 = sqp.tile([128, 4], F32, tag="den1", name=f"den1{nm}")
                nc.vector.tensor_scalar_add(den1[:], ssq128[:], 1.0)
                rcp = sqp.tile([128, 4], F32, tag="rcp", name=f"rcp{nm}")
                nc.vector.reciprocal(rcp[:], den1[:])
                scl128 = sqp.tile([128, 4], BF16, tag="scl128",
                                  name=f"scl128{nm}")
                with nc.allow_low_precision(reason="scale bf16"):
                    if pre != 1.0:
                        sclf = sqp.tile([128, 4], F32, tag="sclf",
                                        name=f"sclf{nm}")
                        nc.vector.tensor_mul(sclf[:], rt[:], rcp[:])
                        nc.vector.tensor_scalar_mul(scl128[:], sclf[:], pre)
                    else:
                        nc.vector.tensor_mul(scl128[:], rt[:], rcp[:])
                scl = sqp.tile([1, B2], BF16, tag="scl", name=f"scl{nm}")
                nc.scalar.dma_start(
                    scl[:].rearrange("u (p f) -> u p f", p=128), scl128[:]
                )
                sbc_ps = sqps.tile([O, B2], F32, tag="sbc", name=f"sbc{nm}")
                nc.tensor.matmul(sbc_ps[:], ones1[:], scl[:],
                                 start=True, stop=True)
                if t == 2:
                    v32 = sqp.tile([O, B2], F32R, tag="v32", name=f"v32{nm}")
                    with nc.allow_low_precision(reason="v out"):
                        nc.vector.tensor_mul(v32[:], s_q[:], sbc_ps[:])
                    nc.sync.dma_start(
                        v_d[32 * q : 32 * (q + 1), :].rearrange(
                            "(cc o) b -> o cc b", cc=2
                        ),
                        v32[:].rearrange("o (cc b) -> o cc b", cc=2),
                    )
                else:
                    vac = vacc[:, 2 * q * B : 2 * (q + 1) * B]
                    with nc.allow_low_precision(reason="v bf16"):
                        if t == 0:
                            nc.vector.tensor_mul(vac, s_q[:], sbc_ps[:])
                        else:
                            vq = sqp.tile([O, B2], BF16, tag="vq",
                                          name=f"vq{nm}")
                            nc.vector.tensor_mul(vq[:], s_q[:], sbc_ps[:])
                            nc.vector.tensor_add(vac, vac, vq[:])
                    # scatter into w4q's diagonal blocks via SBUF-SBUF DMA
                    # (partition-16m targets are illegal for engine APs)
                    engs = [nc.sync, nc.scalar, nc.gpsimd, nc.sync]
                    for r in range(2):
                        c = 2 * q + r
                        src = vacc[:, c * B : (c + 1) * B]
                        for m in range(4):
                            engs[m].dma_start(
                                w4q[O * m : O * (m + 1),
                                    c * 4 * B + m * B : c * 4 * B
                                    + (m + 1) * B],
                                src,
                            )

            # ---------------- phase A: x -> xt_d -> xt  (early, gpsimd DMAs)
            phio_cm = contextlib.ExitStack()
            phio = phio_cm.enter_context(tc.tile_pool(name="phio", bufs=3))
            wtree = phio_cm.enter_context(tc.tile_pool(name="wtree", bufs=2))
            with tc.tile_pool(name="xps", bufs=4, space="PSUM") as xps:
                for bh in range(2):
                    xin = phio.tile([128, IL * K], F32, tag="xin", bufs=2)
                    nc.sync.dma_start(
                        xin[:],
                        x_d[bh * 128 : (bh + 1) * 128].rearrange(
                            "b i k -> b (i k)"
                        ),
                    )
                    xc = phio.tile([128, IL * K], BF16, tag="xc", bufs=1)
                    for qq in range(16):
                        ps = xps.tile([128, 128], F32)
                        nc.tensor.transpose(
                            ps[:], xin[:, qq * 128 : (qq + 1) * 128], ident[:]
                        )
                        nc.scalar.copy(xc[:, qq * 128 : (qq + 1) * 128], ps[:])
                    nc.gpsimd.dma_start(
                        xt_d.rearrange("(q p) b -> p q b", p=128)[
                            :, :, bh * 128 : (bh + 1) * 128
                        ],
                        xc[:].rearrange("p (q b2) -> p q b2", q=16),
                    )
            for h in range(2):
                nc.gpsimd.dma_start(
                    xt[:, h * KB_ : (h + 1) * KB_],
                    xt_d[h * 1024 : (h + 1) * 1024].rearrange(
                        "(p k) b -> p (k b)", k=K
                    ),
                )

            # ---------------- phase B: W pipeline, c-major; s0 and AR0
            # quarters launch as soon as each class pair is done.
            with (
                tc.tile_pool(name="tps", bufs=2, space="PSUM") as tpsp,
                tc.tile_pool(name="s0ps", bufs=2, space="PSUM") as s0psp,
                tc.tile_pool(name="sqps0", bufs=1, space="PSUM") as sqps0,
                tc.tile_pool(name="sq0", bufs=2) as sqp0,
            ):
                s0c_ps = {}
                for c in range(C):
                    s0c_ps[c] = s0psp.tile([O, B], F32, tag="s0",
                                           name=f"s0_{c}")
                    for h in range(2):
                        wt = phio.tile([128, O * D * K], F32, tag="wt", bufs=2)
                        (nc.sync if h == 0 else nc.scalar).dma_start(
                            wt[:],
                            W_d[c, h * 128 : (h + 1) * 128].rearrange(
                                "p o d k -> p (o d k)"
                            ),
                        )
                        # ---- d-reduce into wf[:, (h, :, c, :)]
                        wout = wfv[:, h, c, :, :]  # [p, k, o] contiguous
                        win = wt[:].rearrange("p (o d k) -> p k o d", o=O,
                                              d=D, k=K)
                        # DVE dense-run add tree over d (gpsimd must
                        # stay empty so collective triggers fire early)
                        v4 = wt[:].rearrange("p (o d k) -> p o d k", o=O,
                                             d=D, k=K)
                        a1 = wtree.tile([128, 1024], F32, tag="a1")
                        a1v = a1[:].rearrange("p (o d k) -> p o d k",
                                              o=O, d=8, k=K)
                        nc.vector.tensor_add(a1v, v4[:, :, 0:8, :],
                                             v4[:, :, 8:16, :])
                        a2 = wtree.tile([128, 512], F32, tag="a2")
                        a2v = a2[:].rearrange("p (o d k) -> p o d k",
                                              o=O, d=4, k=K)
                        nc.vector.tensor_add(a2v, a1v[:, :, 0:4, :],
                                             a1v[:, :, 4:8, :])
                        a3 = wtree.tile([128, 256], F32, tag="a3")
                        a3v = a3[:].rearrange("p (o d k) -> p o d k",
                                              o=O, d=2, k=K)
                        nc.vector.tensor_add(a3v, a2v[:, :, 0:2, :],
                                             a2v[:, :, 2:4, :])
                        wout_odk = wout.rearrange(
                            "p k o -> p o k"
                        ).unsqueeze(2)
                        with nc.allow_low_precision(reason="wsum bf16"):
                            nc.vector.tensor_add(
                                wout_odk, a3v[:, :, 0:1, :],
                                a3v[:, :, 1:2, :]
                            )
                        # ---- batched transpose -> Tok4A/Tok4B halves
                        tcol = (h * C + c) * 128
                        tp = tpsp.tile([128, 128], BF16, tag="tp")
                        nc.tensor.transpose(
                            tp[:], wf[:, tcol : tcol + 128], identb[:]
                        )
                        nc.scalar.copy(Tok4A[:, tcol : tcol + 128],
                                       tp[0:64, :])
                        nc.scalar.copy(Tok4B[:, tcol : tcol + 128],
                                       tp[64:128, :])
                        # ---- s0 partials: 8 accumulating matmuls (k)
                        for k in range(K):
                            nc.tensor.matmul(
                                s0c_ps[c][:],
                                wfv[:, h, c, k, :],
                                xt[:, h * KB_ + k * B : h * KB_ + (k + 1) * B],
                                start=(h == 0 and k == 0),
                                stop=(h == 1 and k == K - 1),
                            )
                    if c % 2 == 1:
                        q = c // 2
                        s0q = smallp.tile([O, 2 * B], BF16, tag="s0q",
                                          name=f"s0q{q}", bufs=2)
                        with nc.allow_low_precision(reason="s partial bf16"):
                            nc.scalar.copy(s0q[:, 0:B], s0c_ps[c - 1][:])
                            nc.scalar.copy(s0q[:, B : 2 * B], s0c_ps[c][:])
                        nc.sync.dma_start(cc_in[0][q][:, :], s0q[:])
                        squash_quarter(0, q, 1.0 / C, sqp0, sqps0)

            phio_cm.close()

            # ---------------- routing iterations 1 and 2
            with (
                tc.tile_pool(name="gps", bufs=2, space="PSUM") as gps,
                tc.tile_pool(name="sps", bufs=2, space="PSUM") as spsp,
                tc.tile_pool(name="sqpsi", bufs=1, space="PSUM") as sqpsi,
                tc.tile_pool(name="workp", bufs=1) as workp,
                tc.tile_pool(name="sqi", bufs=2) as sqpi,
            ):
                for it in range(1, 3):
                    # ---- phase 1: G = Tok^T w4; P = x*G; bt = sum_k P
                    for h in range(2):
                        for c in range(C):
                            tcol = (h * C + c) * 128
                            bthc = bt[:, h * CB + c * B : h * CB + (c + 1) * B]
                            ftmp = workp.tile([128, 2 * B], BF16, tag="ftmp",
                                              bufs=3, name=f"ft{it}_{h}_{c}")
                            for kh in range(2):
                                Tok = Tok4A if kh == 0 else Tok4B
                                g_ps = gps.tile([128, 4 * B], F32, tag="g")
                                for hf in range(2):
                                    nc.tensor.matmul(
                                        g_ps[:, hf * 2 * B : (hf + 1) * 2 * B],
                                        Tok[:, tcol : tcol + 128],
                                        w4q[:, c * 4 * B + hf * 2 * B :
                                            c * 4 * B + (hf + 1) * 2 * B],
                                        start=True, stop=True,
                                    )
                                xsl = xt[:, h * KB_ + kh * 4 * B :
                                         h * KB_ + (kh + 1) * 4 * B]
                                # P = x * G : DVE reads PSUM f32 directly on
                                # h1 chunks; Act-copy + Pool mul on h0.
                                phalf = workp.tile([128, 4 * B], BF16,
                                                   tag="phalf", bufs=4,
                                                   name=f"ph{it}_{h}_{c}_{kh}")
                                with nc.allow_low_precision(reason="P bf16"):
                                    if h == 0:
                                        g16 = workp.tile(
                                            [128, 4 * B], BF16, tag="g16",
                                            bufs=3, name=f"g16{it}_{c}_{kh}",
                                        )
                                        nc.scalar.copy(g16[:], g_ps[:])
                                        nc.gpsimd.tensor_mul(
                                            phalf[:], xsl, g16[:]
                                        )
                                    else:
                                        nc.vector.tensor_mul(
                                            phalf[:], xsl, g_ps[:]
                                        )
                                # fold k (4): dense contiguous adds
                                feng = nc.gpsimd if h == 0 else nc.vector
                                f1 = workp.tile(
                                    [128, 2 * B], BF16, tag="f1", bufs=3,
                                    name=f"f1{it}_{h}_{c}_{kh}",
                                )
                                with nc.allow_low_precision(reason="bt bf16"):
                                    feng.tensor_add(
                                        f1[:], phalf[:, 0 : 2 * B],
                                        phalf[:, 2 * B : 4 * B],
                                    )
                                    feng.tensor_add(
                                        ftmp[:, kh * B : (kh + 1) * B],
                                        f1[:, 0:B], f1[:, B : 2 * B],
                                    )
                            with nc.allow_low_precision(reason="bt bf16"):
                                nc.vector.tensor_add(
                                    bthc, ftmp[:, 0:B], ftmp[:, B : 2 * B]
                                )

                        # ---- phase 2 (per h): softmax over c -> e, xs
                        bth = bt[:, h * CB : (h + 1) * CB]
                        bthv = bth.rearrange("p (c b) -> p c b", c=C)
                        eh = e_all[:, h * CB : (h + 1) * CB]
                        ein = bth
                        if True:
                            # max-subtraction: the Act Exp table misbehaves
                            # for large positive inputs, so always subtract
                            m1 = workp.tile([128, 4 * B], BF16, tag="m1",
                                            bufs=2, name=f"m1_{it}_{h}")
                            m1v = m1[:].rearrange("p (c b) -> p c b", c=4)
                            m2 = workp.tile([128, 2 * B], BF16, tag="m2",
                                            bufs=2, name=f"m2_{it}_{h}")
                            m2v = m2[:].rearrange("p (c b) -> p c b", c=2)
                            rmax = workp.tile([128, B], BF16, tag="rmax",
                                              bufs=2, name=f"rm_{it}_{h}")
                            sub = workp.tile([128, CB], BF16, tag="sub",
                                             bufs=2, name=f"sub_{it}_{h}")
                            with nc.allow_low_precision(reason="softmax"):
                                nc.vector.tensor_max(
                                    m1v, bthv[:, 0:4, :], bthv[:, 4:8, :]
                                )
                                nc.vector.tensor_max(
                                    m2v, m1v[:, 0:2, :], m1v[:, 2:4, :]
                                )
                                nc.vector.tensor_max(
                                    rmax[:].unsqueeze(1),
                                    m2v[:, 0:1, :], m2v[:, 1:2, :]
                                )
                                nc.vector.tensor_sub(
                                    sub[:].rearrange("p (c b) -> p c b", c=C),
                                    bthv,
                                    rmax[:].unsqueeze(1)
                                    .broadcast_to([128, C, B]),
                                )
                            ein = sub[:]
                        nc.scalar.activation(
                            eh, ein, mybir.ActivationFunctionType.Exp
                        )
                        ehv = eh.rearrange("p (c b) -> p c b", c=C)
                        # den tree + reciprocal + xs = x * recb
                        d1 = workp.tile([128, 4 * B], BF16, tag="m1",
                                        bufs=2, name=f"d1_{it}_{h}")
                        d1v = d1[:].rearrange("p (c b) -> p c b", c=4)
                        d2 = workp.tile([128, 2 * B], BF16, tag="m2",
                                        bufs=2, name=f"d2_{it}_{h}")
                        d2v = d2[:].rearrange("p (c b) -> p c b", c=2)
                        den = workp.tile([128, B], F32, tag="den",
                                         bufs=2, name=f"den_{it}_{h}")
                        with nc.allow_low_precision(reason="den bf16"):
                            nc.vector.tensor_add(
                                d1v, ehv[:, 0:4, :], ehv[:, 4:8, :]
                            )
                            nc.vector.tensor_add(
                                d2v, d1v[:, 0:2, :], d1v[:, 2:4, :]
                            )
                        nc.vector.tensor_add(
                            den[:].unsqueeze(1),
                            d2v[:, 0:1, :], d2v[:, 1:2, :]
                        )
                        rec = workp.tile([128, B], F32, tag="rec",
                                         bufs=2, name=f"rec_{it}_{h}")
                        nc.vector.reciprocal(rec[:], den[:])
                        recb = workp.tile([128, B], BF16, tag="recb",
                                          bufs=2, name=f"recb_{it}_{h}")
                        with nc.allow_low_precision(reason="recb bf16"):
                            nc.vector.tensor_copy(recb[:], rec[:])
                            nc.vector.tensor_mul(
                                xs[:, h * KB_ : (h + 1) * KB_].rearrange(
                                    "p (k b) -> p k b", k=K
                                ),
                                xt[:, h * KB_ : (h + 1) * KB_].rearrange(
                                    "p (k b) -> p k b", k=K
                                ),
                                recb[:].unsqueeze(1)
                                .broadcast_to([128, K, B]),
                            )

                    # ---- phase 3: y = e_c * xs; s = sum wf^T y; AR quarters
                    sc_ps = {}
                    for c in range(C):
                        sc_ps[c] = spsp.tile([O, B], F32, tag="s",
                                             name=f"s{it}_{c}")
                        for h in range(2):
                            y = workp.tile([128, KB_], BF16, tag="y",
                                           bufs=3, name=f"y{it}_{c}_{h}")
                            yeng = nc.gpsimd if h == 1 else nc.vector
                            with nc.allow_low_precision(reason="y bf16"):
                                yeng.tensor_mul(
                                    y[:].rearrange("p (k b) -> p k b", k=K),
                                    xs[:, h * KB_ : (h + 1) * KB_].rearrange(
                                        "p (k b) -> p k b", k=K
                                    ),
                                    e_all[:, h * CB + c * B :
                                          h * CB + (c + 1) * B]
                                    .unsqueeze(1)
                                    .broadcast_to([128, K, B]),
                                )
                            for k in range(K):
                                nc.tensor.matmul(
                                    sc_ps[c][:],
                                    wfv[:, h, c, k, :],
                                    y[:, k * B : (k + 1) * B],
                                    start=(h == 0 and k == 0),
                                    stop=(h == 1 and k == K - 1),
                                )
                        if c % 2 == 1:
                            q = c // 2
                            sq_sb = smallp.tile([O, 2 * B], BF16,
                                                tag="sq_sb",
                                                name=f"sqsb{it}_{q}", bufs=2)
                            with nc.allow_low_precision(reason="s bf16"):
                                nc.scalar.copy(sq_sb[:, 0:B],
                                               sc_ps[c - 1][:])
                                nc.scalar.copy(sq_sb[:, B : 2 * B],
                                               sc_ps[c][:])
                            nc.sync.dma_start(cc_in[it][q][:, :], sq_sb[:])
                            squash_quarter(it, q, 1.0, sqpi, sqpsi)
    return fixup_multi_waits(nc) if fixup else nc


_NC = None


def kernel(x: np.ndarray, W: np.ndarray, _timings=None) -> np.ndarray:
    global _NC
    x = np.asarray(x, np.float32)
    W = np.asarray(W, np.float32)
    if _NC is None:
        _NC = build_all()
    in_maps = []
    for j in range(NCORES):
        sl = slice(j * IL, (j + 1) * IL)
        in_maps.append(
            {
                "W": np.ascontiguousarray(W[:, 0, sl]),
                "x": np.ascontiguousarray(x[:, sl, :]),
            }
        )
    res = run_bass_kernel_spmd(
        _NC, in_maps, core_ids=list(range(NCORES)),
        trace=_timings is not None,
    )
    if _timings is not None:
        _timings.append(res.exec_time_ns)
    v = res.results[0]["v"].astype(np.float32)  # [(c,o), b]
    return np.ascontiguousarray(
        v.reshape(C, O, B).transpose(2, 0, 1)
    )


# revision 28
# speedup vs baseline: 1.2107x; 1.0616x over previous
"""CapsuleLayer (dynamic routing) on 8 trn2 NeuronCores — v2.

Math: u_hat[b,c,i,o] = sum_{d,k} W[c,0,i,o,d,k] x[b,i,k]
             = sum_k Wsum[c,i,o,k] x[b,i,k],  Wsum = W.sum(d)   (134MB -> 8.4MB)
Sharded over IN_CAPS (i) across 8 cores; only s-partials cross cores.

v2 redesign vs v1 (which was vector-engine bound at 419us):
  * s lives packed by class in PSUM tiles at PE-quadrant-legal bases; the
    AllReduce ships 2-class quarters (bf16, 16KB) that pipeline with compute
    and with the next iteration's start.
  * wf layout [i, (h,k,c,o)] serves all consumers: class-paired s0
    stationaries [128,32] (contiguous), per-class s stationaries [128,16],
    and 8 per-k PE transposes per chunk into TokA/TokB whose k-blocks sit
    at 32-aligned slot bases (16 valid + 16 dead rows per slot).
  * w4 = running sum of squash outputs v, stored slot-replicated
    [128=(4 slots x (16o+16 dead)), (c,b)] via two tiny PE replication
    matmuls per class — so every G matmul has lhsT (Tok slot) and rhs
    (w4 slot) at the same 32-aligned base.
  * d-reduce = one DVE tensor_reduce over d per chunk (Pool add-tree for the
    late classes so DVE is free for overlapped work).
  * k-fold of P = x*G is one DVE tensor_reduce per 4-k half chunk.
  * softmax: iter1 skips max-subtraction (|logit| <~ 65); 1/den is folded
    into x (xs = x * recb) so e stays unnormalized and ct is never formed.
  * h-major iteration order: softmax(h0) overlaps the P-phase of h1.
"""

import contextlib
import sys
import types

import numpy as np
import ml_dtypes  # noqa: F401


def _install_ntff_shim():
    try:
        import antenv.axon_hooks  # noqa: F401

        return
    except Exception:
        pass
    import ctypes

    mod = types.ModuleType("antenv.axon_hooks")
    holder = [None, False]

    def set_axon_ntff_profile_hook(h):
        holder[0], holder[1] = h, True

    def _make_hook():
        try:
            lib = ctypes.CDLL("/opt/axon/libaxon_pjrt.so")
        except OSError:
            return None
        if not hasattr(lib, "axon_start_nrt_profile"):
            return None
        lib.axon_start_nrt_profile.argtypes = [
            ctypes.POINTER(ctypes.c_int64),
            ctypes.c_size_t,
        ]
        lib.axon_start_nrt_profile.restype = ctypes.c_int64
        lib.axon_stop_nrt_profile.argtypes = [ctypes.c_char_p]
        lib.axon_stop_nrt_profile.restype = ctypes.c_int64

        @contextlib.contextmanager
        def _hook(output_dir, device_ids):
            import jax

            jax.devices()
            if device_ids:
                ids = (ctypes.c_int64 * len(device_ids))(*device_ids)
                rc = lib.axon_start_nrt_profile(ids, len(device_ids))
            else:
                rc = lib.axon_start_nrt_profile(None, 0)
            if rc != 0:
                raise RuntimeError(f"axon_start_nrt_profile rc={rc}")
            try:
                yield
            finally:
                n = lib.axon_stop_nrt_profile(str(output_dir).encode())
                print(
                    f"profile: {n} file(s) written to {output_dir}",
                    file=sys.stderr,
                )

        return _hook

    def get_axon_ntff_profile_hook():
        if not holder[1]:
            holder[0], holder[1] = _make_hook(), True
        return holder[0]

    mod.set_axon_ntff_profile_hook = set_axon_ntff_profile_hook
    mod.get_axon_ntff_profile_hook = get_axon_ntff_profile_hook
    sys.modules["antenv.axon_hooks"] = mod


try:
    _install_ntff_shim()
except Exception:
    pass

import concourse.bass as bass
import concourse.mybir as mybir
import concourse.tile as tile
from concourse import masks
from concourse.bass_utils import run_bass_kernel_spmd
from bass_rust import ScopedClock

# ---------------------------------------------------------------- constants
C, I, O, D, K, B = 8, 2048, 16, 16, 8, 256
NCORES = 8
IL = I // NCORES          # 256 i's per core
F32 = mybir.dt.float32
F32R = mybir.dt.float32r
BF16 = mybir.dt.bfloat16
KB_ = K * B               # 2048 cols per h-block of x
CB = C * B                # 2048

# ------------------------------------------------- tile tail-drain workaround
_MAX_WAITS = 1


def _patched_drain_and_barrier(self, tick_clock, wait_clock):
    nc = self.nc
    drain_inst = nc.sync.drain()
    wait_clock.add_sem_waits(
        drain_inst.ins, ScopedClock({None: tick_clock.global_clock})
    )
    si = drain_inst.ins.sync_info
    if si is not None and si.on_wait and len(si.on_wait) > _MAX_WAITS:
        waits = list(si.on_wait)
        si.on_wait = waits[:_MAX_WAITS]
        for i in range(_MAX_WAITS, len(waits), _MAX_WAITS):
            extra = nc.sync.drain()
            extra.ins.sync_info = mybir.SyncInfo(
                on_wait=waits[i : i + _MAX_WAITS], on_update=[]
            )
    nc.all_engine_barrier()
    assert self.sems is not None
    popped = nc._tile_sem_poison_stack.pop()
    assert popped is self._sem_poison
    nc.clear_and_free_semaphores(list(self.sems.allocated().values()))
    nc.all_engine_barrier()


tile.TileContext._drain_and_barrier = _patched_drain_and_barrier

_fix_ctr = [0]


def fixup_multi_waits(nc):
    """walrus in this toolchain accepts at most one sem wait per instruction;
    hoist extra waits onto same-engine drains placed just before."""
    for f in nc.m.functions:
        for bb in f.blocks:
            out = []
            for inst in bb.instructions:
                si = inst.sync_info
                if si is not None and si.on_wait and len(si.on_wait) > _MAX_WAITS:
                    waits = list(si.on_wait)
                    for i in range(0, len(waits) - _MAX_WAITS, _MAX_WAITS):
                        _fix_ctr[0] += 1
                        d = mybir.InstDrain(
                            name=f"waitsplit_{_fix_ctr[0]}", ins=[], outs=[]
                        )
                        d.engine = inst.engine
                        d.sync_info = mybir.SyncInfo(
                            on_wait=waits[i : i + _MAX_WAITS], on_update=[]
                        )
                        out.append(d)
                    si.on_wait = waits[len(waits) - _MAX_WAITS :]
                out.append(inst)
            bb.instructions[:] = out
    return nc


def build_all(fixup=True):
    nc = bass.Bass("TRN2", target_bir_lowering=False, debug=False,
                   num_devices=NCORES)
    W_d = nc.dram_tensor("W", [C, IL, O, D, K], F32, kind="ExternalInput").ap()
    x_d = nc.dram_tensor("x", [B, IL, K], F32, kind="ExternalInput").ap()
    # v packed [(c,o)=128, b=256]
    v_d = nc.dram_tensor("v", [C * O, B], F32R, kind="ExternalOutput").ap()
    xt_d = nc.dram_tensor("xt", [IL * K, B], BF16).ap()
    # per-(iter, quarter) collective buffers: [16o, (2 classes, b)] bf16
    cc_in = [[nc.dram_tensor(f"cc_in{t}_{q}", [O, 2 * B], BF16).ap()
              for q in range(4)] for t in range(3)]
    cc_out = [[nc.dram_tensor(f"cc_out{t}_{q}", [O, 2 * B], BF16).ap()
               for q in range(4)] for t in range(3)]

    with tile.TileContext(nc) as tc:
        with (
            tc.tile_pool(name="const", bufs=1) as constp,
            tc.tile_pool(name="persist", bufs=1) as pers,
            tc.tile_pool(name="small", bufs=4) as smallp,
        ):
            # ---------------- constants
            ident = constp.tile([128, 128], F32)
            masks.make_identity(nc, ident[:])
            identb = constp.tile([128, 128], BF16)
            with nc.allow_low_precision(reason="identity copy"):
                nc.vector.tensor_copy(identb[:], ident[:])
            # squash reducers: sum over o (partition dim) and broadcast back
            ones16 = constp.tile([O, 1], BF16)
            nc.vector.memset(ones16[:], 1.0)
            ones1 = constp.tile([1, O], BF16)
            nc.vector.memset(ones1[:], 1.0)
            # ---------------- persistent state
            # xt: [i(h-blocked 128p), (h, k, b)] bf16
            xt = pers.tile([128, 2 * KB_], BF16)
            # wf: d-reduced W, [i, (h, c, k, o)] bf16
            wf = pers.tile([128, 2 * K * C * O], BF16)
            # Tok4{A,B}: [(4k, o)=64, (c, h, i128)] bf16 (G stationaries;
            # A holds k=0..3, B k=4..7)
            Tok4A = pers.tile([64, C * 2 * 128], BF16)
            Tok4B = pers.tile([64, C * 2 * 128], BF16)
            # w4q: block-diagonal mover for G: [64=(kg,o), (c, kmod4, b)];
            # diagonal [16,256] blocks hold the running v sum, rest is 0.
            # (diag blocks at partition 16m are written via SBUF-SBUF DMA —
            # engine APs must start at 32-aligned partitions, DMAs need not.)
            w4q = pers.tile([64, C * 4 * B], BF16)
            nc.vector.memset(w4q[:], 0.0)
            # vacc: running v sum, [16o, (c, b)]
            vacc = pers.tile([O, C * B], BF16)
            # logits bt: [i, (h, c, b)] bf16
            bt = pers.tile([128, 2 * CB], BF16)
            # e = exp(bt - max) (unnormalized), same layout
            e_all = pers.tile([128, 2 * CB], BF16)
            # xs = x * (1/den), [i, (h, k, b)] bf16
            xs = pers.tile([128, 2 * KB_], BF16)

            wfv = wf[:].rearrange("p (h c k o) -> p h c k o", h=2, c=C, k=K,
                                  o=O)

            # ---------- AllReduce + squash on a 2-class quarter ----------
            # s arrives as [16o, (cc, b)] bf16.  scale = sqrt(ss)/(1+ss).
            def squash_quarter(t, q, pre, sqp, sqps):
                nm = f"{t}_{q}"
                B2 = 2 * B
                nc.gpsimd.collective_compute(
                    "AllReduce",
                    mybir.AluOpType.add,
                    replica_groups=[list(range(NCORES))],
                    ins=[cc_in[t][q].opt()],
                    outs=[cc_out[t][q].opt()],
                )
                s_q = sqp.tile([O, B2], BF16, tag="s_q", name=f"sq{nm}")
                nc.sync.dma_start(s_q[:], cc_out[t][q][:, :])
                sq2 = sqp.tile([O, B2], BF16, tag="sq2", name=f"sq2{nm}")
                with nc.allow_low_precision(reason="square bf16"):
                    nc.scalar.activation(
                        sq2[:], s_q[:], mybir.ActivationFunctionType.Square,
                        scale=pre,
                    )
                ssq_ps = sqps.tile([1, B2], F32, tag="ssq", name=f"ssqp{nm}")
                nc.tensor.matmul(ssq_ps[:], ones16[:], sq2[:],
                                 start=True, stop=True)
                # scale = pre * sqrt(ss)/(1+ss); single-partition DVE
                # ops run one lane, so bounce [1,512] -> [128,4] via DMA.
                ssq_row = sqp.tile([1, B2], F32R, tag="ssq_row",
                                   name=f"ssqr{nm}")
                nc.scalar.copy(ssq_row[:], ssq_ps[:])
                ssq128 = sqp.tile([128, 4], F32R, tag="ssq128",
                                  name=f"ssq128{nm}")
                nc.sync.dma_start(
                    ssq128[:], ssq_row[:].rearrange("u (p f) -> u p f", p=128)
                )
                rt = sqp.tile([128, 4], F32, tag="rt", name=f"rt{nm}")
                nc.scalar.sqrt(rt[:], ssq128[:])
                den1 = sqp.tile([128, 4], F32, tag="den1", name=f"den1{nm}")
                nc.vector.tensor_scalar_add(den1[:], ssq128[:], 1.0)
                rcp = sqp.tile([128, 4], F32, tag="rcp", name=f"rcp{nm}")
                nc.vector.reciprocal(rcp[:], den1[:])
                scl128 = sqp.tile([128, 4], BF16, tag="scl128",
                                  name=f"scl128{nm}")
                with nc.allow_low_precision(reason="scale bf16"):
                    if pre != 1.0:
                        sclf = sqp.tile([128, 4], F32, tag="sclf",
                                        name=f"sclf{nm}")
                        nc.vector.tensor_mul(sclf[:], rt[:], rcp[:])
                        nc.vector.tensor_scalar_mul(scl128[:], sclf[:], pre)
                    else:
                        nc.vector.tensor_mul(scl128[:], rt[:], rcp[:])
                scl = sqp.tile([1, B2], BF16, tag="scl", name=f"scl{nm}")
                nc.scalar.dma_start(
                    scl[:].rearrange("u (p f) -> u p f", p=128), scl128[:]
                )
                sbc_ps = sqps.tile([O, B2], F32, tag="sbc", name=f"sbc{nm}")
                nc.tensor.matmul(sbc_ps[:], ones1[:], scl[:],
                                 start=True, stop=True)
                if t == 2:
                    v32 = sqp.tile([O, B2], F32R, tag="v32", name=f"v32{nm}")
                    with nc.allow_low_precision(reason="v out"):
                        nc.vector.tensor_mul(v32[:], s_q[:], sbc_ps[:])
                    nc.sync.dma_start(
                        v_d[32 * q : 32 * (q + 1), :].rearrange(
                            "(cc o) b -> o cc b", cc=2
                        ),
                        v32[:].rearrange("o (cc b) -> o cc b", cc=2),
                    )
                else:
                    vac = vacc[:, 2 * q * B : 2 * (q + 1) * B]
                    with nc.allow_low_precision(reason="v bf16"):
                        if t == 0:
                            nc.vector.tensor_mul(vac, s_q[:], sbc_ps[:])
                        else:
                            vq = sqp.tile([O, B2], BF16, tag="vq",
                                          name=f"vq{nm}")
                            nc.vector.tensor_mul(vq[:], s_q[:], sbc_ps[:])
                            nc.vector.tensor_add(vac, vac, vq[:])
                    # scatter into w4q's diagonal blocks via SBUF-SBUF DMA
                    # (partition-16m targets are illegal for engine APs)
                    engs = [nc.sync, nc.scalar, nc.sync, nc.scalar]
                    for r in range(2):
                        c = 2 * q + r
                        src = vacc[:, c * B : (c + 1) * B]
                        for m in range(4):
                            engs[m].dma_start(
                                w4q[O * m : O * (m + 1),
                                    c * 4 * B + m * B : c * 4 * B
                                    + (m + 1) * B],
                                src,
                            )

            # ---------------- phase A: x -> xt_d -> xt  (early, gpsimd DMAs)
            phio_cm = contextlib.ExitStack()
            phio = phio_cm.enter_context(tc.tile_pool(name="phio", bufs=3))
            wtree = phio_cm.enter_context(tc.tile_pool(name="wtree", bufs=2))
            with tc.tile_pool(name="xps", bufs=4, space="PSUM") as xps:
                for bh in range(2):
                    xin = phio.tile([128, IL * K], F32, tag="xin", bufs=2)
                    nc.sync.dma_start(
                        xin[:],
                        x_d[bh * 128 : (bh + 1) * 128].rearrange(
                            "b i k -> b (i k)"
                        ),
                    )
                    xc = phio.tile([128, IL * K], BF16, tag="xc", bufs=1)
                    for qq in range(16):
                        ps = xps.tile([128, 128], F32)
                        nc.tensor.transpose(
                            ps[:], xin[:, qq * 128 : (qq + 1) * 128], ident[:]
                        )
                        nc.scalar.copy(xc[:, qq * 128 : (qq + 1) * 128], ps[:])
                    nc.scalar.dma_start(
                        xt_d.rearrange("(q p) b -> p q b", p=128)[
                            :, :, bh * 128 : (bh + 1) * 128
                        ],
                        xc[:].rearrange("p (q b2) -> p q b2", q=16),
                    )
            for h in range(2):
                nc.scalar.dma_start(
                    xt[:, h * KB_ : (h + 1) * KB_],
                    xt_d[h * 1024 : (h + 1) * 1024].rearrange(
                        "(p k) b -> p (k b)", k=K
                    ),
                )

            # ---------------- phase B: W pipeline, c-major; s0 and AR0
            # quarters launch as soon as each class pair is done.
            with (
                tc.tile_pool(name="tps", bufs=2, space="PSUM") as tpsp,
                tc.tile_pool(name="s0ps", bufs=2, space="PSUM") as s0psp,
                tc.tile_pool(name="sqps0", bufs=1, space="PSUM") as sqps0,
                tc.tile_pool(name="sq0", bufs=2) as sqp0,
            ):
                s0c_ps = {}
                for c in range(C):
                    s0c_ps[c] = s0psp.tile([O, B], F32, tag="s0",
                                           name=f"s0_{c}")
                    for h in range(2):
                        wt = phio.tile([128, O * D * K], F32, tag="wt", bufs=2)
                        (nc.sync if h == 0 else nc.scalar).dma_start(
                            wt[:],
                            W_d[c, h * 128 : (h + 1) * 128].rearrange(
                                "p o d k -> p (o d k)"
                            ),
                        )
                        # ---- d-reduce into wf[:, (h, :, c, :)]
                        wout = wfv[:, h, c, :, :]  # [p, k, o] contiguous
                        win = wt[:].rearrange("p (o d k) -> p k o d", o=O,
                                              d=D, k=K)
                        # DVE dense-run add tree over d (gpsimd must
                        # stay empty so collective triggers fire early)
                        v4 = wt[:].rearrange("p (o d k) -> p o d k", o=O,
                                             d=D, k=K)
                        a1 = wtree.tile([128, 1024], F32, tag="a1")
                        a1v = a1[:].rearrange("p (o d k) -> p o d k",
                                              o=O, d=8, k=K)
                        nc.vector.tensor_add(a1v, v4[:, :, 0:8, :],
                                             v4[:, :, 8:16, :])
                        a2 = wtree.tile([128, 512], F32, tag="a2")
                        a2v = a2[:].rearrange("p (o d k) -> p o d k",
                                              o=O, d=4, k=K)
                        nc.vector.tensor_add(a2v, a1v[:, :, 0:4, :],
                                             a1v[:, :, 4:8, :])
                        a3 = wtree.tile([128, 256], F32, tag="a3")
                        a3v = a3[:].rearrange("p (o d k) -> p o d k",
                                              o=O, d=2, k=K)
                        nc.vector.tensor_add(a3v, a2v[:, :, 0:2, :],
                                             a2v[:, :, 2:4, :])
                        wout_odk = wout.rearrange(
                            "p k o -> p o k"
                        ).unsqueeze(2)
                        with nc.allow_low_precision(reason="wsum bf16"):
                            nc.vector.tensor_add(
                                wout_odk, a3v[:, :, 0:1, :],
                                a3v[:, :, 1:2, :]
                            )
                        # ---- batched transpose -> Tok4A/Tok4B halves
                        tcol = (h * C + c) * 128
                        tp = tpsp.tile([128, 128], BF16, tag="tp")
                        nc.tensor.transpose(
                            tp[:], wf[:, tcol : tcol + 128], identb[:]
                        )
                        nc.scalar.copy(Tok4A[:, tcol : tcol + 128],
                                       tp[0:64, :])
                        nc.scalar.copy(Tok4B[:, tcol : tcol + 128],
                                       tp[64:128, :])
                        # ---- s0 partials: 8 accumulating matmuls (k)
                        for k in range(K):
                            nc.tensor.matmul(
                                s0c_ps[c][:],
                                wfv[:, h, c, k, :],
                                xt[:, h * KB_ + k * B : h * KB_ + (k + 1) * B],
                                start=(h == 0 and k == 0),
                                stop=(h == 1 and k == K - 1),
                            )
                    if c % 2 == 1:
                        q = c // 2
                        s0q = smallp.tile([O, 2 * B], BF16, tag="s0q",
                                          name=f"s0q{q}", bufs=2)
                        with nc.allow_low_precision(reason="s partial bf16"):
                            nc.scalar.copy(s0q[:, 0:B], s0c_ps[c - 1][:])
                            nc.scalar.copy(s0q[:, B : 2 * B], s0c_ps[c][:])
                        nc.sync.dma_start(cc_in[0][q][:, :], s0q[:])
                        squash_quarter(0, q, 1.0 / C, sqp0, sqps0)

            phio_cm.close()

            # ---------------- routing iterations 1 and 2
            with (
                tc.tile_pool(name="gps", bufs=2, space="PSUM") as gps,
                tc.tile_pool(name="sps", bufs=2, space="PSUM") as spsp,
                tc.tile_pool(name="sqpsi", bufs=1, space="PSUM") as sqpsi,
                tc.tile_pool(name="workp", bufs=1) as workp,
                tc.tile_pool(name="sqi", bufs=2) as sqpi,
            ):
                for it in range(1, 3):
                    # ---- phase 1: G = Tok^T w4; P = x*G; bt = sum_k P
                    for h in range(2):
                        for c in range(C):
                            tcol = (h * C + c) * 128
                            bthc = bt[:, h * CB + c * B : h * CB + (c + 1) * B]
                            ftmp = workp.tile([128, 2 * B], BF16, tag="ftmp",
                                              bufs=3, name=f"ft{it}_{h}_{c}")
                            for kh in range(2):
                                Tok = Tok4A if kh == 0 else Tok4B
                                g_ps = gps.tile([128, 4 * B], F32, tag="g")
                                for hf in range(2):
                                    nc.tensor.matmul(
                                        g_ps[:, hf * 2 * B : (hf + 1) * 2 * B],
                                        Tok[:, tcol : tcol + 128],
                                        w4q[:, c * 4 * B + hf * 2 * B :
                                            c * 4 * B + (hf + 1) * 2 * B],
                                        start=True, stop=True,
                                    )
                                xsl = xt[:, h * KB_ + kh * 4 * B :
                                         h * KB_ + (kh + 1) * 4 * B]
                                # P = x * G : DVE reads PSUM f32 directly on
                                # h1 chunks; Act-copy + Pool mul on h0.
                                phalf = workp.tile([128, 4 * B], BF16,
                                                   tag="phalf", bufs=4,
                                                   name=f"ph{it}_{h}_{c}_{kh}")
                                with nc.allow_low_precision(reason="P bf16"):
                                    if c % 2 == 0:
                                        g16 = workp.tile(
                                            [128, 4 * B], BF16, tag="g16",
                                            bufs=3, name=f"g16{it}_{c}_{kh}",
                                        )
                                        nc.scalar.copy(g16[:], g_ps[:])
                                        nc.gpsimd.tensor_mul(
                                            phalf[:], xsl, g16[:]
                                        )
                                    else:
                                        nc.vector.tensor_mul(
                                            phalf[:], xsl, g_ps[:]
                                        )
                                # fold k (4): dense contiguous adds
                                feng = nc.gpsimd if c % 2 == 0 else nc.vector
                                f1 = workp.tile(
                                    [128, 2 * B], BF16, tag="f1", bufs=3,
                                    name=f"f1{it}_{h}_{c}_{kh}",
                                )
                                with nc.allow_low_precision(reason="bt bf16"):
                                    feng.tensor_add(
                                        f1[:], phalf[:, 0 : 2 * B],
                                        phalf[:, 2 * B : 4 * B],
                                    )
                                    feng.tensor_add(
                                        ftmp[:, kh * B : (kh + 1) * B],
                                        f1[:, 0:B], f1[:, B : 2 * B],
                                    )
                            with nc.allow_low_precision(reason="bt bf16"):
                                nc.vector.tensor_add(
                                    bthc, ftmp[:, 0:B], ftmp[:, B : 2 * B]
                                )

                        # ---- phase 2 (per h): softmax over c -> e, xs
                        bth = bt[:, h * CB : (h + 1) * CB]
                        bthv = bth.rearrange("p (c b) -> p c b", c=C)
                        eh = e_all[:, h * CB : (h + 1) * CB]
                        ein = bth
                        if True:
                            # max-subtraction: the Act Exp table misbehaves
                            # for large positive inputs, so always subtract
                            m1 = workp.tile([128, 4 * B], BF16, tag="m1",
                                            bufs=2, name=f"m1_{it}_{h}")
                            m1v = m1[:].rearrange("p (c b) -> p c b", c=4)
                            m2 = workp.tile([128, 2 * B], BF16, tag="m2",
                                            bufs=2, name=f"m2_{it}_{h}")
                            m2v = m2[:].rearrange("p (c b) -> p c b", c=2)
                            rmax = workp.tile([128, B], BF16, tag="rmax",
                                              bufs=2, name=f"rm_{it}_{h}")
                            sub = workp.tile([128, CB], BF16, tag="sub",
                                             bufs=2, name=f"sub_{it}_{h}")
                            with nc.allow_low_precision(reason="softmax"):
                                nc.vector.tensor_max(
                                    m1v, bthv[:, 0:4, :], bthv[:, 4:8, :]
                                )
                                nc.vector.tensor_max(
                                    m2v, m1v[:, 0:2, :], m1v[:, 2:4, :]
                                )
                                nc.vector.tensor_max(
                                    rmax[:].unsqueeze(1),
                                    m2v[:, 0:1, :], m2v[:, 1:2, :]
                                )
                                nc.vector.tensor_sub(
                                    sub[:].rearrange("p (c b) -> p c b", c=C),
                                    bthv,
                                    rmax[:].unsqueeze(1)
                                    .broadcast_to([128, C, B]),
                                )
                            ein = sub[:]
                        nc.scalar.activation(
                            eh, ein, mybir.ActivationFunctionType.Exp
                        )
                        ehv = eh.rearrange("p (c b) -> p c b", c=C)
                        # den tree + reciprocal + xs = x * recb
                        d1 = workp.tile([128, 4 * B], BF16, tag="m1",
                                        bufs=2, name=f"d1_{it}_{h}")
                        d1v = d1[:].rearrange("p (c b) -> p c b", c=4)
                        d2 = workp.tile([128, 2 * B], BF16, tag="m2",
                                        bufs=2, name=f"d2_{it}_{h}")
                        d2v = d2[:].rearrange("p (c b) -> p c b", c=2)
                        den = workp.tile([128, B], F32, tag="den",
                                         bufs=2, name=f"den_{it}_{h}")
                        with nc.allow_low_precision(reason="den bf16"):
                            nc.vector.tensor_add(
                                d1v, ehv[:, 0:4, :], ehv[:, 4:8, :]
                            )
                            nc.vector.tensor_add(
                                d2v, d1v[:, 0:2, :], d1v[:, 2:4, :]
                            )
                        nc.vector.tensor_add(
                            den[:].unsqueeze(1),
                            d2v[:, 0:1, :], d2v[:, 1:2, :]
                        )
                        rec = workp.tile([128, B], F32, tag="rec",
                                         bufs=2, name=f"rec_{it}_{h}")
                        nc.vector.reciprocal(rec[:], den[:])
                        recb = workp.tile([128, B], BF16, tag="recb",
                                          bufs=2, name=f"recb_{it}_{h}")
                        with nc.allow_low_precision(reason="recb bf16"):
                            nc.vector.tensor_copy(recb[:], rec[:])
                            nc.vector.tensor_mul(
                                xs[:, h * KB_ : (h + 1) * KB_].rearrange(
                                    "p (k b) -> p k b", k=K
                                ),
                                xt[:, h * KB_ : (h + 1) * KB_].rearrange(
                                    "p (k b) -> p k b", k=K
                                ),
                                recb[:].unsqueeze(1)
                                .broadcast_to([128, K, B]),
                            )

                    # ---- phase 3: y = e_c * xs; s = sum wf^T y; AR quarters
                    sc_ps = {}
                    for c in range(C):
                        sc_ps[c] = spsp.tile([O, B], F32, tag="s",
                                             name=f"s{it}_{c}")
                        for h in range(2):
                            y = workp.tile([128, KB_], BF16, tag="y",
                                           bufs=3, name=f"y{it}_{c}_{h}")
                            yeng = nc.gpsimd if c % 2 == 0 else nc.vector
                            with nc.allow_low_precision(reason="y bf16"):
                                yeng.tensor_mul(
                                    y[:].rearrange("p (k b) -> p k b", k=K),
                                    xs[:, h * KB_ : (h + 1) * KB_].rearrange(
                                        "p (k b) -> p k b", k=K
                                    ),
                                    e_all[:, h * CB + c * B :
                                          h * CB + (c + 1) * B]
                                    .unsqueeze(1)
                                    .broadcast_to([128, K, B]),
                                )
                            for k in range(K):
                                nc.tensor.matmul(
                                    sc_ps[c][:],
                                    wfv[:, h, c, k, :],
                                    y[:, k * B : (k + 1) * B],
                                    start=(h == 0 and k == 0),
                                    stop=(h == 1 and k == K - 1),
                                )
                        if c % 2 == 1:
                            q = c // 2
                            sq_sb = smallp.tile([O, 2 * B], BF16,
                                                tag="sq_sb",
                                                name=f"sqsb{it}_{q}", bufs=2)
                            with nc.allow_low_precision(reason="s bf16"):
                                nc.scalar.copy(sq_sb[:, 0:B],
                                               sc_ps[c - 1][:])
                                nc.scalar.copy(sq_sb[:, B : 2 * B],
                                               sc_ps[c][:])
                            nc.sync.dma_start(cc_in[it][q][:, :], sq_sb[:])
                            squash_quarter(it, q, 1.0, sqpi, sqpsi)
    return fixup_multi_waits(nc) if fixup else nc


_NC = None


def kernel(x: np.ndarray, W: np.ndarray, _timings=None) -> np.ndarray:
    global _NC
    x = np.asarray(x, np.float32)
    W = np.asarray(W, np.float32)
    if _NC is None:
        _NC = build_all()
    in_maps = []
    for j in range(NCORES):
        sl = slice(j * IL, (j + 1) * IL)
        in_maps.append(
            {
                "W": np.ascontiguousarray(W[:, 0, sl]),
                "x": np.ascontiguousarray(x[:, sl, :]),
            }
        )
    res = run_bass_kernel_spmd(
        _NC, in_maps, core_ids=list(range(NCORES)),
        trace=_timings is not None,
    )
    if _timings is not None:
        _timings.append(res.exec_time_ns)
    v = res.results[0]["v"].astype(np.float32)  # [(c,o), b]
    return np.ascontiguousarray(
        v.reshape(C, O, B).transpose(2, 0, 1)
    )


# revision 30
# speedup vs baseline: 1.2424x; 1.0261x over previous
"""CapsuleLayer (dynamic routing) on 8 trn2 NeuronCores — v2.

Math: u_hat[b,c,i,o] = sum_{d,k} W[c,0,i,o,d,k] x[b,i,k]
             = sum_k Wsum[c,i,o,k] x[b,i,k],  Wsum = W.sum(d)   (134MB -> 8.4MB)
Sharded over IN_CAPS (i) across 8 cores; only s-partials cross cores.

v2 redesign vs v1 (which was vector-engine bound at 419us):
  * s lives packed by class in PSUM tiles at PE-quadrant-legal bases; the
    AllReduce ships 2-class quarters (bf16, 16KB) that pipeline with compute
    and with the next iteration's start.
  * wf layout [i, (h,k,c,o)] serves all consumers: class-paired s0
    stationaries [128,32] (contiguous), per-class s stationaries [128,16],
    and 8 per-k PE transposes per chunk into TokA/TokB whose k-blocks sit
    at 32-aligned slot bases (16 valid + 16 dead rows per slot).
  * w4 = running sum of squash outputs v, stored slot-replicated
    [128=(4 slots x (16o+16 dead)), (c,b)] via two tiny PE replication
    matmuls per class — so every G matmul has lhsT (Tok slot) and rhs
    (w4 slot) at the same 32-aligned base.
  * d-reduce = one DVE tensor_reduce over d per chunk (Pool add-tree for the
    late classes so DVE is free for overlapped work).
  * k-fold of P = x*G is one DVE tensor_reduce per 4-k half chunk.
  * softmax: iter1 skips max-subtraction (|logit| <~ 65); 1/den is folded
    into x (xs = x * recb) so e stays unnormalized and ct is never formed.
  * h-major iteration order: softmax(h0) overlaps the P-phase of h1.
"""

import contextlib
import sys
import types

import numpy as np
import ml_dtypes  # noqa: F401


def _install_ntff_shim():
    try:
        import antenv.axon_hooks  # noqa: F401

        return
    except Exception:
        pass
    import ctypes

    mod = types.ModuleType("antenv.axon_hooks")
    holder = [None, False]

    def set_axon_ntff_profile_hook(h):
        holder[0], holder[1] = h, True

    def _make_hook():
        try:
            lib = ctypes.CDLL("/opt/axon/libaxon_pjrt.so")
        except OSError:
            return None
        if not hasattr(lib, "axon_start_nrt_profile"):
            return None
        lib.axon_start_nrt_profile.argtypes = [
            ctypes.POINTER(ctypes.c_int64),
            ctypes.c_size_t,
        ]
        lib.axon_start_nrt_profile.restype = ctypes.c_int64
        lib.axon_stop_nrt_profile.argtypes = [ctypes.c_char_p]
        lib.axon_stop_nrt_profile.restype = ctypes.c_int64

        @contextlib.contextmanager
        def _hook(output_dir, device_ids):
            import jax

            jax.devices()
            if device_ids:
                ids = (ctypes.c_int64 * len(device_ids))(*device_ids)
                rc = lib.axon_start_nrt_profile(ids, len(device_ids))
            else:
                rc = lib.axon_start_nrt_profile(None, 0)
            if rc != 0:
                raise RuntimeError(f"axon_start_nrt_profile rc={rc}")
            try:
                yield
            finally:
                n = lib.axon_stop_nrt_profile(str(output_dir).encode())
                print(
                    f"profile: {n} file(s) written to {output_dir}",
                    file=sys.stderr,
                )

        return _hook

    def get_axon_ntff_profile_hook():
        if not holder[1]:
            holder[0], holder[1] = _make_hook(), True
        return holder[0]

    mod.set_axon_ntff_profile_hook = set_axon_ntff_profile_hook
    mod.get_axon_ntff_profile_hook = get_axon_ntff_profile_hook
    sys.modules["antenv.axon_hooks"] = mod


try:
    _install_ntff_shim()
except Exception:
    pass

import concourse.bass as bass
import concourse.mybir as mybir
import concourse.tile as tile
from concourse import masks
from concourse.bass_utils import run_bass_kernel_spmd
from bass_rust import ScopedClock

# ---------------------------------------------------------------- constants
C, I, O, D, K, B = 8, 2048, 16, 16, 8, 256
NCORES = 8
IL = I // NCORES          # 256 i's per core
F32 = mybir.dt.float32
F32R = mybir.dt.float32r
BF16 = mybir.dt.bfloat16
KB_ = K * B               # 2048 cols per h-block of x
CB = C * B                # 2048

# ------------------------------------------------- tile tail-drain workaround
_MAX_WAITS = 1


def _patched_drain_and_barrier(self, tick_clock, wait_clock):
    nc = self.nc
    drain_inst = nc.sync.drain()
    wait_clock.add_sem_waits(
        drain_inst.ins, ScopedClock({None: tick_clock.global_clock})
    )
    si = drain_inst.ins.sync_info
    if si is not None and si.on_wait and len(si.on_wait) > _MAX_WAITS:
        waits = list(si.on_wait)
        si.on_wait = waits[:_MAX_WAITS]
        for i in range(_MAX_WAITS, len(waits), _MAX_WAITS):
            extra = nc.sync.drain()
            extra.ins.sync_info = mybir.SyncInfo(
                on_wait=waits[i : i + _MAX_WAITS], on_update=[]
            )
    nc.all_engine_barrier()
    assert self.sems is not None
    popped = nc._tile_sem_poison_stack.pop()
    assert popped is self._sem_poison
    nc.clear_and_free_semaphores(list(self.sems.allocated().values()))
    nc.all_engine_barrier()


tile.TileContext._drain_and_barrier = _patched_drain_and_barrier

_fix_ctr = [0]


def fixup_multi_waits(nc):
    """walrus in this toolchain accepts at most one sem wait per instruction;
    hoist extra waits onto same-engine drains placed just before."""
    for f in nc.m.functions:
        for bb in f.blocks:
            out = []
            for inst in bb.instructions:
                si = inst.sync_info
                if si is not None and si.on_wait and len(si.on_wait) > _MAX_WAITS:
                    waits = list(si.on_wait)
                    for i in range(0, len(waits) - _MAX_WAITS, _MAX_WAITS):
                        _fix_ctr[0] += 1
                        d = mybir.InstDrain(
                            name=f"waitsplit_{_fix_ctr[0]}", ins=[], outs=[]
                        )
                        d.engine = inst.engine
                        d.sync_info = mybir.SyncInfo(
                            on_wait=waits[i : i + _MAX_WAITS], on_update=[]
                        )
                        out.append(d)
                    si.on_wait = waits[len(waits) - _MAX_WAITS :]
                out.append(inst)
            bb.instructions[:] = out
    return nc


def build_all(fixup=True):
    nc = bass.Bass("TRN2", target_bir_lowering=False, debug=False,
                   num_devices=NCORES)
    W_d = nc.dram_tensor("W", [C, IL, O, D, K], F32, kind="ExternalInput").ap()
    x_d = nc.dram_tensor("x", [B, IL, K], F32, kind="ExternalInput").ap()
    # v packed [(c,o)=128, b=256]
    v_d = nc.dram_tensor("v", [C * O, B], F32R, kind="ExternalOutput").ap()
    xt_d = nc.dram_tensor("xt", [IL * K, B], BF16).ap()
    # per-(iter, quarter) collective buffers: [16o, (2 classes, b)] bf16
    cc_in = [[nc.dram_tensor(f"cc_in{t}_{q}", [O, 2 * B], BF16).ap()
              for q in range(4)] for t in range(3)]
    cc_out = [[nc.dram_tensor(f"cc_out{t}_{q}", [O, 2 * B], BF16).ap()
               for q in range(4)] for t in range(3)]
    cc_warm_in = nc.dram_tensor("cc_warm_in", [1, 16], BF16).ap()
    cc_warm_out = nc.dram_tensor("cc_warm_out", [1, 16], BF16).ap()

    with tile.TileContext(nc) as tc:
        with (
            tc.tile_pool(name="const", bufs=1) as constp,
            tc.tile_pool(name="persist", bufs=1) as pers,
            tc.tile_pool(name="small", bufs=4) as smallp,
        ):
            # ---------------- constants
            ident = constp.tile([128, 128], F32)
            masks.make_identity(nc, ident[:])
            identb = constp.tile([128, 128], BF16)
            with nc.allow_low_precision(reason="identity copy"):
                nc.vector.tensor_copy(identb[:], ident[:])
            # squash reducers: sum over o (partition dim) and broadcast back
            ones16 = constp.tile([O, 1], BF16)
            nc.vector.memset(ones16[:], 1.0)
            ones1 = constp.tile([1, O], BF16)
            nc.vector.memset(ones1[:], 1.0)
            # ---------------- persistent state
            # xt: [i(h-blocked 128p), (h, k, b)] bf16
            xt = pers.tile([128, 2 * KB_], BF16)
            # wf: d-reduced W, [i, (h, c, k, o)] bf16
            wf = pers.tile([128, 2 * K * C * O], BF16)
            # Tok4{A,B}: [(4k, o)=64, (c, h, i128)] bf16 (G stationaries;
            # A holds k=0..3, B k=4..7)
            Tok4A = pers.tile([64, C * 2 * 128], BF16)
            Tok4B = pers.tile([64, C * 2 * 128], BF16)
            # w4q: block-diagonal mover for G: [64=(kg,o), (c, kmod4, b)];
            # diagonal [16,256] blocks hold the running v sum, rest is 0.
            # (diag blocks at partition 16m are written via SBUF-SBUF DMA —
            # engine APs must start at 32-aligned partitions, DMAs need not.)
            w4q = pers.tile([64, C * 4 * B], BF16)
            nc.vector.memset(w4q[:], 0.0)
            # vacc: running v sum, [16o, (c, b)]
            vacc = pers.tile([O, C * B], BF16)
            # logits bt: [i, (h, c, b)] bf16
            bt = pers.tile([128, 2 * CB], BF16)
            # e = exp(bt - max) (unnormalized), same layout
            e_all = pers.tile([128, 2 * CB], BF16)
            # xs = x * (1/den), [i, (h, k, b)] bf16
            xs = pers.tile([128, 2 * KB_], BF16)

            wfv = wf[:].rearrange("p (h c k o) -> p h c k o", h=2, c=C, k=K,
                                  o=O)

            # ---------- AllReduce + squash on a 2-class quarter ----------
            # s arrives as [16o, (cc, b)] bf16.  scale = sqrt(ss)/(1+ss).
            # Trigger (gpsimd collective) is issued as soon as the quarter's
            # partials are in cc_in; the tail is issued interleaved with the
            # NEXT iteration's chunk loop so no engine head-blocks on the
            # AllReduce completion.
            def trigger_ar(t, q):
                nc.gpsimd.collective_compute(
                    "AllReduce",
                    mybir.AluOpType.add,
                    replica_groups=[list(range(NCORES))],
                    ins=[cc_in[t][q].opt()],
                    outs=[cc_out[t][q].opt()],
                )

            def squash_tail(t, q, pre, sqp, sqps):
                nm = f"{t}_{q}"
                B2 = 2 * B
                s_q = sqp.tile([O, B2], BF16, tag="s_q", name=f"sq{nm}")
                nc.sync.dma_start(s_q[:], cc_out[t][q][:, :])
                sq2 = sqp.tile([O, B2], BF16, tag="sq2", name=f"sq2{nm}")
                with nc.allow_low_precision(reason="square bf16"):
                    nc.scalar.activation(
                        sq2[:], s_q[:], mybir.ActivationFunctionType.Square,
                        scale=pre,
                    )
                sq_ps = sqps.tile([O, B2], F32, tag="sqps",
                                  name=f"sqps{nm}")
                ssq_ps = sq_ps[0:1, :]
                nc.tensor.matmul(ssq_ps, ones16[:], sq2[:],
                                 start=True, stop=True)
                # scale = pre * sqrt(ss)/(1+ss); single-partition DVE
                # ops run one lane, so bounce [1,512] -> [128,4] via DMA.
                ssq_row = sqp.tile([1, B2], F32R, tag="ssq_row",
                                   name=f"ssqr{nm}")
                nc.scalar.copy(ssq_row[:], ssq_ps)
                ssq128 = sqp.tile([128, 4], F32R, tag="ssq128",
                                  name=f"ssq128{nm}")
                nc.sync.dma_start(
                    ssq128[:], ssq_row[:].rearrange("u (p f) -> u p f", p=128)
                )
                rt = sqp.tile([128, 4], F32, tag="rt", name=f"rt{nm}")
                nc.scalar.sqrt(rt[:], ssq128[:])
                den1 = sqp.tile([128, 4], F32, tag="den1", name=f"den1{nm}")
                nc.vector.tensor_scalar_add(den1[:], ssq128[:], 1.0)
                rcp = sqp.tile([128, 4], F32, tag="rcp", name=f"rcp{nm}")
                nc.vector.reciprocal(rcp[:], den1[:])
                scl128 = sqp.tile([128, 4], BF16, tag="scl128",
                                  name=f"scl128{nm}")
                with nc.allow_low_precision(reason="scale bf16"):
                    if pre != 1.0:
                        sclf = sqp.tile([128, 4], F32, tag="sclf",
                                        name=f"sclf{nm}")
                        nc.vector.tensor_mul(sclf[:], rt[:], rcp[:])
                        nc.vector.tensor_scalar_mul(scl128[:], sclf[:], pre)
                    else:
                        nc.vector.tensor_mul(scl128[:], rt[:], rcp[:])
                scl = sqp.tile([1, B2], BF16, tag="scl", name=f"scl{nm}")
                nc.scalar.dma_start(
                    scl[:].rearrange("u (p f) -> u p f", p=128), scl128[:]
                )
                sbc_ps = sq_ps
                nc.tensor.matmul(sbc_ps[:], ones1[:], scl[:],
                                 start=True, stop=True)
                if t == 2:
                    v32 = sqp.tile([O, B2], F32R, tag="v32", name=f"v32{nm}")
                    with nc.allow_low_precision(reason="v out"):
                        nc.vector.tensor_mul(v32[:], s_q[:], sbc_ps[:])
                    nc.sync.dma_start(
                        v_d[32 * q : 32 * (q + 1), :].rearrange(
                            "(cc o) b -> o cc b", cc=2
                        ),
                        v32[:].rearrange("o (cc b) -> o cc b", cc=2),
                    )
                else:
                    vac = vacc[:, 2 * q * B : 2 * (q + 1) * B]
                    with nc.allow_low_precision(reason="v bf16"):
                        if t == 0:
                            nc.vector.tensor_mul(vac, s_q[:], sbc_ps[:])
                        else:
                            vq = sqp.tile([O, B2], BF16, tag="vq",
                                          name=f"vq{nm}")
                            nc.vector.tensor_mul(vq[:], s_q[:], sbc_ps[:])
                            nc.vector.tensor_add(vac, vac, vq[:])
                    # scatter into w4q's diagonal blocks via SBUF-SBUF DMA
                    # (partition-16m targets are illegal for engine APs)
                    engs = [nc.sync, nc.scalar, nc.sync, nc.scalar]
                    for r in range(2):
                        c = 2 * q + r
                        src = vacc[:, c * B : (c + 1) * B]
                        for m in range(4):
                            engs[m].dma_start(
                                w4q[O * m : O * (m + 1),
                                    c * 4 * B + m * B : c * 4 * B
                                    + (m + 1) * B],
                                src,
                            )

            # warmup collective: absorbs the cross-core barrier and CC
            # cold-start while phase A/B compute proceeds
            warm = smallp.tile([1, 16], BF16, tag="warm", bufs=1)
            nc.vector.memset(warm[:], 0.0)
            nc.sync.dma_start(cc_warm_in[:, :], warm[:])
            nc.gpsimd.collective_compute(
                "AllReduce",
                mybir.AluOpType.add,
                replica_groups=[list(range(NCORES))],
                ins=[cc_warm_in.opt()],
                outs=[cc_warm_out.opt()],
            )

            # ---------------- phase A: x -> xt_d -> xt  (early, gpsimd DMAs)
            phio_cm = contextlib.ExitStack()
            phio = phio_cm.enter_context(tc.tile_pool(name="phio", bufs=3))
            wtree = phio_cm.enter_context(tc.tile_pool(name="wtree", bufs=2))
            with tc.tile_pool(name="xps", bufs=4, space="PSUM") as xps:
                for bh in range(2):
                    xin = phio.tile([128, IL * K], F32, tag="xin", bufs=2)
                    nc.sync.dma_start(
                        xin[:],
                        x_d[bh * 128 : (bh + 1) * 128].rearrange(
                            "b i k -> b (i k)"
                        ),
                    )
                    xc = phio.tile([128, IL * K], BF16, tag="xc", bufs=1)
                    for qq in range(16):
                        ps = xps.tile([128, 128], F32)
                        nc.tensor.transpose(
                            ps[:], xin[:, qq * 128 : (qq + 1) * 128], ident[:]
                        )
                        nc.scalar.copy(xc[:, qq * 128 : (qq + 1) * 128], ps[:])
                    nc.scalar.dma_start(
                        xt_d.rearrange("(q p) b -> p q b", p=128)[
                            :, :, bh * 128 : (bh + 1) * 128
                        ],
                        xc[:].rearrange("p (q b2) -> p q b2", q=16),
                    )
            for h in range(2):
                nc.scalar.dma_start(
                    xt[:, h * KB_ : (h + 1) * KB_],
                    xt_d[h * 1024 : (h + 1) * 1024].rearrange(
                        "(p k) b -> p (k b)", k=K
                    ),
                )

            # ---------------- phase B: W pipeline, c-major; s0 and AR0
            # quarters launch as soon as each class pair is done.
            with (
                tc.tile_pool(name="tps", bufs=2, space="PSUM") as tpsp,
                tc.tile_pool(name="s0ps", bufs=2, space="PSUM") as s0psp,
                tc.tile_pool(name="sqps0", bufs=1, space="PSUM") as sqps0,
                tc.tile_pool(name="sq0", bufs=2) as sqp0,
            ):
                s0c_ps = {}
                for c in range(C):
                    s0c_ps[c] = s0psp.tile([O, B], F32, tag="s0",
                                           name=f"s0_{c}")
                    for h in range(2):
                        wt = phio.tile([128, O * D * K], F32, tag="wt", bufs=2)
                        (nc.sync if h == 0 else nc.scalar).dma_start(
                            wt[:],
                            W_d[c, h * 128 : (h + 1) * 128].rearrange(
                                "p o d k -> p (o d k)"
                            ),
                        )
                        # ---- d-reduce into wf[:, (h, :, c, :)]
                        wout = wfv[:, h, c, :, :]  # [p, k, o] contiguous
                        win = wt[:].rearrange("p (o d k) -> p k o d", o=O,
                                              d=D, k=K)
                        # DVE dense-run add tree over d (gpsimd must
                        # stay empty so collective triggers fire early)
                        v4 = wt[:].rearrange("p (o d k) -> p o d k", o=O,
                                             d=D, k=K)
                        a1 = wtree.tile([128, 1024], F32, tag="a1")
                        a1v = a1[:].rearrange("p (o d k) -> p o d k",
                                              o=O, d=8, k=K)
                        nc.vector.tensor_add(a1v, v4[:, :, 0:8, :],
                                             v4[:, :, 8:16, :])
                        a2 = wtree.tile([128, 512], F32, tag="a2")
                        a2v = a2[:].rearrange("p (o d k) -> p o d k",
                                              o=O, d=4, k=K)
                        nc.vector.tensor_add(a2v, a1v[:, :, 0:4, :],
                                             a1v[:, :, 4:8, :])
                        a3 = wtree.tile([128, 256], F32, tag="a3")
                        a3v = a3[:].rearrange("p (o d k) -> p o d k",
                                              o=O, d=2, k=K)
                        nc.vector.tensor_add(a3v, a2v[:, :, 0:2, :],
                                             a2v[:, :, 2:4, :])
                        wout_odk = wout.rearrange(
                            "p k o -> p o k"
                        ).unsqueeze(2)
                        with nc.allow_low_precision(reason="wsum bf16"):
                            nc.vector.tensor_add(
                                wout_odk, a3v[:, :, 0:1, :],
                                a3v[:, :, 1:2, :]
                            )
                        # ---- batched transpose -> Tok4A/Tok4B halves
                        tcol = (h * C + c) * 128
                        tp = tpsp.tile([128, 128], BF16, tag="tp")
                        nc.tensor.transpose(
                            tp[:], wf[:, tcol : tcol + 128], identb[:]
                        )
                        nc.scalar.copy(Tok4A[:, tcol : tcol + 128],
                                       tp[0:64, :])
                        nc.scalar.copy(Tok4B[:, tcol : tcol + 128],
                                       tp[64:128, :])
                        # ---- s0 partials: 8 accumulating matmuls (k)
                        for k in range(K):
                            nc.tensor.matmul(
                                s0c_ps[c][:],
                                wfv[:, h, c, k, :],
                                xt[:, h * KB_ + k * B : h * KB_ + (k + 1) * B],
                                start=(h == 0 and k == 0),
                                stop=(h == 1 and k == K - 1),
                            )
                    if c % 2 == 1:
                        q = c // 2
                        s0q = smallp.tile([O, 2 * B], BF16, tag="s0q",
                                          name=f"s0q{q}", bufs=2)
                        with nc.allow_low_precision(reason="s partial bf16"):
                            nc.scalar.copy(s0q[:, 0:B], s0c_ps[c - 1][:])
                            nc.scalar.copy(s0q[:, B : 2 * B], s0c_ps[c][:])
                        nc.sync.dma_start(cc_in[0][q][:, :], s0q[:])
                        trigger_ar(0, q)

            phio_cm.close()

            # ---------------- routing iterations 1 and 2
            with (
                tc.tile_pool(name="gps", bufs=3, space="PSUM") as gps,
                tc.tile_pool(name="sps", bufs=1, space="PSUM") as spsp,
                tc.tile_pool(name="sqpsi", bufs=1, space="PSUM") as sqpsi,
                tc.tile_pool(name="workp", bufs=1) as workp,
                tc.tile_pool(name="sqi", bufs=2) as sqpi,
            ):
                for it in range(1, 3):
                    # ---- phase 1: G = Tok^T w4; P = x*G; bt = sum_k P
                    for h in range(2):
                        for c in range(C):
                            if h == 0 and c % 2 == 0:
                                squash_tail(it - 1, c // 2,
                                            1.0 / C if it == 1 else 1.0,
                                            sqpi, sqpsi)
                            tcol = (h * C + c) * 128
                            bthc = bt[:, h * CB + c * B : h * CB + (c + 1) * B]
                            ftmp = workp.tile([128, 2 * B], BF16, tag="ftmp",
                                              bufs=3, name=f"ft{it}_{h}_{c}")
                            for kh in range(2):
                                Tok = Tok4A if kh == 0 else Tok4B
                                g_ps = gps.tile([128, 4 * B], F32, tag="g")
                                for hf in range(2):
                                    nc.tensor.matmul(
                                        g_ps[:, hf * 2 * B : (hf + 1) * 2 * B],
                                        Tok[:, tcol : tcol + 128],
                                        w4q[:, c * 4 * B + hf * 2 * B :
                                            c * 4 * B + (hf + 1) * 2 * B],
                                        start=True, stop=True,
                                    )
                                xsl = xt[:, h * KB_ + kh * 4 * B :
                                         h * KB_ + (kh + 1) * 4 * B]
                                # P = x * G : DVE reads PSUM f32 directly on
                                # h1 chunks; Act-copy + Pool mul on h0.
                                phalf = workp.tile([128, 4 * B], BF16,
                                                   tag="phalf", bufs=4,
                                                   name=f"ph{it}_{h}_{c}_{kh}")
                                with nc.allow_low_precision(reason="P bf16"):
                                    if c % 2 == 0:
                                        g16 = workp.tile(
                                            [128, 4 * B], BF16, tag="g16",
                                            bufs=3, name=f"g16{it}_{c}_{kh}",
                                        )
                                        nc.scalar.copy(g16[:], g_ps[:])
                                        nc.gpsimd.tensor_mul(
                                            phalf[:], xsl, g16[:]
                                        )
                                    else:
                                        nc.vector.tensor_mul(
                                            phalf[:], xsl, g_ps[:]
                                        )
                                # fold k (4): dense contiguous adds
                                feng = nc.gpsimd if c % 2 == 0 else nc.vector
                                f1 = workp.tile(
                                    [128, 2 * B], BF16, tag="f1", bufs=3,
                                    name=f"f1{it}_{h}_{c}_{kh}",
                                )
                                with nc.allow_low_precision(reason="bt bf16"):
                                    feng.tensor_add(
                                        f1[:], phalf[:, 0 : 2 * B],
                                        phalf[:, 2 * B : 4 * B],
                                    )
                                    feng.tensor_add(
                                        ftmp[:, kh * B : (kh + 1) * B],
                                        f1[:, 0:B], f1[:, B : 2 * B],
                                    )
                            with nc.allow_low_precision(reason="bt bf16"):
                                nc.vector.tensor_add(
                                    bthc, ftmp[:, 0:B], ftmp[:, B : 2 * B]
                                )

                        # ---- phase 2 (per h): softmax over c -> e, xs
                        bth = bt[:, h * CB : (h + 1) * CB]
                        bthv = bth.rearrange("p (c b) -> p c b", c=C)
                        eh = e_all[:, h * CB : (h + 1) * CB]
                        ein = bth
                        if True:
                            # max-subtraction: the Act Exp table misbehaves
                            # for large positive inputs, so always subtract
                            m1 = workp.tile([128, 4 * B], BF16, tag="m1",
                                            bufs=2, name=f"m1_{it}_{h}")
                            m1v = m1[:].rearrange("p (c b) -> p c b", c=4)
                            m2 = workp.tile([128, 2 * B], BF16, tag="m2",
                                            bufs=2, name=f"m2_{it}_{h}")
                            m2v = m2[:].rearrange("p (c b) -> p c b", c=2)
                            rmax = workp.tile([128, B], BF16, tag="rmax",
                                              bufs=2, name=f"rm_{it}_{h}")
                            sub = workp.tile([128, CB], BF16, tag="sub",
                                             bufs=2, name=f"sub_{it}_{h}")
                            with nc.allow_low_precision(reason="softmax"):
                                nc.vector.tensor_max(
                                    m1v, bthv[:, 0:4, :], bthv[:, 4:8, :]
                                )
                                nc.vector.tensor_max(
                                    m2v, m1v[:, 0:2, :], m1v[:, 2:4, :]
                                )
                                nc.vector.tensor_max(
                                    rmax[:].unsqueeze(1),
                                    m2v[:, 0:1, :], m2v[:, 1:2, :]
                                )
                                nc.vector.tensor_sub(
                                    sub[:].rearrange("p (c b) -> p c b", c=C),
                                    bthv,
                                    rmax[:].unsqueeze(1)
                                    .broadcast_to([128, C, B]),
                                )
                            ein = sub[:]
                        nc.scalar.activation(
                            eh, ein, mybir.ActivationFunctionType.Exp
                        )
                        ehv = eh.rearrange("p (c b) -> p c b", c=C)
                        # den tree + reciprocal + xs = x * recb
                        d1 = workp.tile([128, 4 * B], BF16, tag="m1",
                                        bufs=2, name=f"d1_{it}_{h}")
                        d1v = d1[:].rearrange("p (c b) -> p c b", c=4)
                        d2 = workp.tile([128, 2 * B], BF16, tag="m2",
                                        bufs=2, name=f"d2_{it}_{h}")
                        d2v = d2[:].rearrange("p (c b) -> p c b", c=2)
                        den = workp.tile([128, B], F32, tag="den",
                                         bufs=2, name=f"den_{it}_{h}")
                        with nc.allow_low_precision(reason="den bf16"):
                            nc.vector.tensor_add(
                                d1v, ehv[:, 0:4, :], ehv[:, 4:8, :]
                            )
                            nc.vector.tensor_add(
                                d2v, d1v[:, 0:2, :], d1v[:, 2:4, :]
                            )
                        nc.vector.tensor_add(
                            den[:].unsqueeze(1),
                            d2v[:, 0:1, :], d2v[:, 1:2, :]
                        )
                        rec = workp.tile([128, B], F32, tag="rec",
                                         bufs=2, name=f"rec_{it}_{h}")
                        nc.vector.reciprocal(rec[:], den[:])
                        recb = workp.tile([128, B], BF16, tag="recb",
                                          bufs=2, name=f"recb_{it}_{h}")
                        with nc.allow_low_precision(reason="recb bf16"):
                            nc.vector.tensor_copy(recb[:], rec[:])
                            nc.vector.tensor_mul(
                                xs[:, h * KB_ : (h + 1) * KB_].rearrange(
                                    "p (k b) -> p k b", k=K
                                ),
                                xt[:, h * KB_ : (h + 1) * KB_].rearrange(
                                    "p (k b) -> p k b", k=K
                                ),
                                recb[:].unsqueeze(1)
                                .broadcast_to([128, K, B]),
                            )

                    # ---- phase 3: y = e_c * xs; s = sum wf^T y; AR quarters
                    sc_ps = {}
                    for c in range(C):
                        sc_ps[c] = spsp.tile([O, B], F32, tag="s",
                                             name=f"s{it}_{c}")
                        for h in range(2):
                            y = workp.tile([128, KB_], BF16, tag="y",
                                           bufs=3, name=f"y{it}_{c}_{h}")
                            yeng = nc.gpsimd if c % 2 == 0 else nc.vector
                            with nc.allow_low_precision(reason="y bf16"):
                                yeng.tensor_mul(
                                    y[:].rearrange("p (k b) -> p k b", k=K),
                                    xs[:, h * KB_ : (h + 1) * KB_].rearrange(
                                        "p (k b) -> p k b", k=K
                                    ),
                                    e_all[:, h * CB + c * B :
                                          h * CB + (c + 1) * B]
                                    .unsqueeze(1)
                                    .broadcast_to([128, K, B]),
                                )
                            for k in range(K):
                                nc.tensor.matmul(
                                    sc_ps[c][:],
                                    wfv[:, h, c, k, :],
                                    y[:, k * B : (k + 1) * B],
                                    start=(h == 0 and k == 0),
                                    stop=(h == 1 and k == K - 1),
                                )
                        if c % 2 == 1:
                            q = c // 2
                            sq_sb = smallp.tile([O, 2 * B], BF16,
                                                tag="sq_sb",
                                                name=f"sqsb{it}_{q}", bufs=2)
                            with nc.allow_low_precision(reason="s bf16"):
                                nc.scalar.copy(sq_sb[:, 0:B],
                                               sc_ps[c - 1][:])
                                nc.scalar.copy(sq_sb[:, B : 2 * B],
                                               sc_ps[c][:])
                            nc.sync.dma_start(cc_in[it][q][:, :], sq_sb[:])
                            trigger_ar(it, q)
                            if it == 2:
                                squash_tail(2, q, 1.0, sqpi, sqpsi)
    return fixup_multi_waits(nc) if fixup else nc


_NC = None


def kernel(x: np.ndarray, W: np.ndarray, _timings=None) -> np.ndarray:
    global _NC
    x = np.asarray(x, np.float32)
    W = np.asarray(W, np.float32)
    if _NC is None:
        _NC = build_all()
    in_maps = []
    for j in range(NCORES):
        sl = slice(j * IL, (j + 1) * IL)
        in_maps.append(
            {
                "W": np.ascontiguousarray(W[:, 0, sl]),
                "x": np.ascontiguousarray(x[:, sl, :]),
            }
        )
    res = run_bass_kernel_spmd(
        _NC, in_maps, core_ids=list(range(NCORES)),
        trace=_timings is not None,
    )
    if _timings is not None:
        _timings.append(res.exec_time_ns)
    v = res.results[0]["v"].astype(np.float32)  # [(c,o), b]
    return np.ascontiguousarray(
        v.reshape(C, O, B).transpose(2, 0, 1)
    )
